# revision 33
# baseline (speedup 1.0000x reference)
"""Trainium2 Bass kernel for a 16-head attention block with 2D axial RoPE.

Strategy: pure data-parallel over batch (32 batches -> 4 per NeuronCore),
bf16 compute with an fp8 (DoubleRow) QKV q/k projection, feature-major
("transposed") layouts throughout:
  - q/k stay feature-major from the QKV projection (fp8 weights x256 with
    the 1/256 folded into the rope tables; errors attenuate through
    softmax); v is produced token-major directly in bf16.
  - RoPE via two elementwise muls + a pair-swap permutation matmul;
    feature tiles processed in pairs sharing [128,2,512] PSUM tiles so
    every elementwise op covers two tiles in one instruction.
  - scoresT[m,n] per head (keys on partitions): row-packed K=64 matmul
    pairs via tile_position writing the two halves of one PSUM pair
    tile; si1+si2 share one bank; exp runs as two activations per head
    pair; block-causal mask applied as one small 0/1-mask multiply over
    query columns 0:85; softmax without max subtraction; per-query sums
    via an appended ones-column on v; normalization via DMA-broadcast
    reciprocal sums + one bf16 multiply per head.
  - proj maps back to token-major; bf16 output DMA.
"""
import sys, os
sys.path.insert(0, "/opt/trn_rl_repo")
import numpy as np
import ml_dtypes

B, NTOK, DIM, H, HD = 32, 341, 1024, 16, 64
NCORES, BPC = 8, 4          # cores, batches per core
NP = 344                    # padded tokens per batch (bf16 pair aligned)
T = BPC * NP                # 1376 tokens per core
SCALES = [1, 2, 4, 8, 16]
PT_SEQ_LEN, THETA = 16, 10000.0
ROPE_DIM = HD // 2
MSL = [(0, 128), (128, 128), (256, 85)]   # m/token slices per batch
NW = 341 - 85               # si>0 query window size (= 256, one half-bank)
FP8_QK = False              # fp8 DoubleRow q/k projection (fails 2e-2 gate)
W8SCALE = 256.0
BF16 = ml_dtypes.bfloat16
F8 = ml_dtypes.float8_e4m3fn

_cache = {}


def _rope_tables():
    inv = 1.0 / (THETA ** (np.arange(0, ROPE_DIM, 2, dtype=np.float64) / ROPE_DIM))
    cos_list, sin_list = [], []
    for s in SCALES:
        t = np.arange(s, dtype=np.float64) / s * PT_SEQ_LEN
        f = np.outer(t, inv)
        f = np.repeat(f, 2, axis=-1)
        fy = np.broadcast_to(f[:, None, :], (s, s, ROPE_DIM))
        fx = np.broadcast_to(f[None, :, :], (s, s, ROPE_DIM))
        ff = np.concatenate([fy, fx], axis=-1).reshape(s * s, HD)
        cos_list.append(np.cos(ff))
        sin_list.append(np.sin(ff))
    cos = np.concatenate(cos_list, axis=0).astype(np.float32)  # [341, 64]
    sin = np.concatenate(sin_list, axis=0).astype(np.float32)
    return cos, sin


def _host_tables():
    cos, sin = _rope_tables()               # [341, 64]
    # sin2: sign pattern for rotate_half: q'[2i] = q[2i]c - q[2i+1]s ...
    sin2 = sin.copy()
    sin2[:, 0::2] = -sin[:, 0::2]
    # sinP[e] = sin2[e^1] (so that (PI @ (q*sinP))[d] = q[d^1]*sin2[d])
    sinP = np.empty_like(sin2)
    sinP[:, 0::2] = sin2[:, 1::2]
    sinP[:, 1::2] = sin2[:, 0::2]
    cosT = np.zeros((HD, NP), np.float32)
    sinPT = np.zeros((HD, NP), np.float32)
    cosT[:, :NTOK] = cos.T
    sinPT[:, :NTOK] = sinP.T
    cos128 = np.vstack([cosT, cosT])        # [128, NP] two heads per tile
    sinP128 = np.vstack([sinPT, sinPT])
    qs = (1.0 / np.sqrt(HD)) / (W8SCALE if FP8_QK else 1.0)
    ks = 1.0 / (W8SCALE if FP8_QK else 1.0)
    # tabs: [cosq, cosq, sinq, sinq, cosk, cosk, sink, sink] (f-pair duplicated)
    blocks = [cos128 * qs, cos128 * qs, sinP128 * qs, sinP128 * qs,
              cos128 * ks, cos128 * ks, sinP128 * ks, sinP128 * ks]
    tabs = np.concatenate(blocks, axis=1)   # [128, 8*NP]
    # consts: PI | I128 | ones | bcm2 (duplicated per head-pair)
    PI = np.zeros((128, 128), np.float32)
    for d in range(128):
        PI[d ^ 1, d] = 1.0
    I128 = np.eye(128, dtype=np.float32)
    ones = np.ones((128, 128), dtype=np.float32)
    seg = np.concatenate([np.full(s * s, i, dtype=np.int64) for i, s in enumerate(SCALES)])
    bcm = np.ones((128, 96), dtype=np.float32)
    bcm[:, 0:85] = (seg[:128, None] <= seg[None, :85]).astype(np.float32)
    consts = np.concatenate([PI, I128, ones, bcm, bcm], axis=1)  # [128, 576]
    return tabs.astype(BF16), consts.astype(BF16)


def _build(mask_mode, use_qkv_bias):
    import concourse.bass as bass
    import concourse.bacc as bacc
    import concourse.tile as tile
    from concourse import mybir

    f32, bf16 = mybir.dt.float32, mybir.dt.bfloat16
    f8 = mybir.dt.float8e4
    AF = mybir.ActivationFunctionType
    DR = mybir.MatmulPerfMode.DoubleRow
    nc = bacc.Bacc("TRN2", target_bir_lowering=False, debug=False)

    xt_d = nc.dram_tensor("xt", [DIM, T], bf16, kind="ExternalInput")
    if FP8_QK:
        xt8_d = nc.dram_tensor("xt8", [DIM, T], f8, kind="ExternalInput")
        wqk8_d = nc.dram_tensor("wqk8", [DIM, 2048], f8, kind="ExternalInput")
    else:
        wqk_d = nc.dram_tensor("wqk", [DIM, 2048], bf16, kind="ExternalInput")
    wv_d = nc.dram_tensor("wv", [DIM, 1024], bf16, kind="ExternalInput")
    wp_d = nc.dram_tensor("wp", [DIM, 1024], bf16, kind="ExternalInput")
    tabs_d = nc.dram_tensor("tabs", [128, 8 * NP], bf16, kind="ExternalInput")
    consts_d = nc.dram_tensor("consts", [128, 576], bf16, kind="ExternalInput")
    general = mask_mode == "general"
    if general:
        maskm_d = nc.dram_tensor("maskm", [128, 3 * NP], bf16, kind="ExternalInput")
    if use_qkv_bias:
        qb_d = nc.dram_tensor("qb", [128, 16 * NP], bf16, kind="ExternalInput")
        vb_d = nc.dram_tensor("vb", [1, 1024], bf16, kind="ExternalInput")
    out_d = nc.dram_tensor("out", [BPC * NTOK, DIM], bf16, kind="ExternalOutput")

    bc = mask_mode == "bc"
    # ex free-dim layout: bc packs si1+si2 as one 512-col block
    EXW = NP + 2 * 256 if bc else 3 * NP
    EXOFF = [0, NP, NP + 256] if bc else [0, NP, 2 * NP]

    with tile.TileContext(nc) as tc, \
         nc.allow_low_precision(reason="bf16/fp8 qk; rel gate 2e-2"):
        with tc.tile_pool(name="res", bufs=1) as res, \
             tc.tile_pool(name="vp", bufs=9) as vpool, \
             tc.tile_pool(name="qkp", bufs=2) as qkpool, \
             tc.tile_pool(name="ro", bufs=2) as ropool, \
             tc.tile_pool(name="ex", bufs=2) as expool, \
             tc.tile_pool(name="asb", bufs=2) as asbp, \
             tc.tile_pool(name="st", bufs=1) as stpool, \
             tc.tile_pool(name="at", bufs=2) as atpool, \
             tc.tile_pool(name="prb", bufs=1) as prbp, \
             tc.tile_pool(name="ys", bufs=2) as yspool, \
             tc.tile_pool(name="dr", bufs=1, space="DRAM") as drp, \
             tc.tile_pool(name="sa", bufs=2, space="PSUM") as sap, \
             tc.tile_pool(name="av", bufs=1, space="PSUM") as avp, \
             tc.tile_pool(name="hf", bufs=2, space="PSUM") as hfp:

            # ---- resident loads (order matters: v(0..2) cover the rest) ----
            xt = res.tile([128, 8, T], bf16)
            if FP8_QK:
                xt8 = res.tile([128, 8, T], f8)
                wqk = res.tile([128, 8, 2048], f8)
            else:
                wqk = res.tile([128, 8, 2048], bf16)
            wv = res.tile([128, 8, 1024], bf16)
            wp = res.tile([128, 8, 1024], bf16)
            for c in range(8):
                nc.sync.dma_start(wv[:, c, :], wv_d[c * 128:(c + 1) * 128, :])
                nc.sync.dma_start(xt[:, c, 0:NP], xt_d[c * 128:(c + 1) * 128, 0:NP])
            for b in range(1, 3):
                for c in range(8):
                    nc.sync.dma_start(xt[:, c, b * NP:(b + 1) * NP],
                                      xt_d[c * 128:(c + 1) * 128, b * NP:(b + 1) * NP])
            tabs = res.tile([128, 8, NP], bf16)
            nc.sync.dma_start(tabs[:], tabs_d[:])
            consts = res.tile([128, 576], bf16)
            nc.sync.dma_start(consts[:], consts_d[:])
            for c in range(8):
                if FP8_QK:
                    nc.sync.dma_start(wqk[:, c, :], wqk8_d[c * 128:(c + 1) * 128, :])
                    nc.sync.dma_start(xt8[:, c, :], xt8_d[c * 128:(c + 1) * 128, :])
                else:
                    nc.sync.dma_start(wqk[:, c, :], wqk_d[c * 128:(c + 1) * 128, :])
            for c in range(8):
                nc.sync.dma_start(wp[:, c, :], wp_d[c * 128:(c + 1) * 128, :])
            for c in range(8):
                nc.sync.dma_start(xt[:, c, 3 * NP:T],
                                  xt_d[c * 128:(c + 1) * 128, 3 * NP:T])
            if general:
                maskm = res.tile([128, 3, NP], bf16)
                nc.sync.dma_start(maskm[:], maskm_d[:])
            if use_qkv_bias:
                qb = res.tile([128, 16, NP], bf16)
                nc.sync.dma_start(qb[:], qb_d[:])
                vb = res.tile([1, 1024], bf16)
                nc.sync.dma_start(vb[:], vb_d[:])

            PI = consts[:, 0:128]
            I128 = consts[:, 128:256]
            bcm2 = consts[:, 384:576]  # [128, 2, 96] view below

            def sa2(name):
                return sap.tile([128, 2, 512], f32, tag="sa", name=name)

            vt = {}     # b -> [v_s tiles per slice]
            att = {}    # b -> att tile
            prb = {}    # b -> broadcast reciprocal sums [64, 16, NP]
            asba = {}   # b -> asb_all [128, 16, NP]

            def emit_v(b):
                boff = b * NP
                vts = []
                for s, (t0, tsz) in enumerate(MSL):
                    v_s = vpool.tile([128, 16, 65], bf16, name="v_s")
                    for half in range(2):
                        pv = hfp.tile([128, 512], f32, tag="hf", name="pv")
                        for c in range(8):
                            nc.tensor.matmul(
                                pv[0:tsz, :],
                                lhsT=xt[:, c, boff + t0: boff + t0 + tsz],
                                rhs=wv[:, c, half * 512:(half + 1) * 512],
                                start=(c == 0), stop=(c == 7 and not use_qkv_bias))
                        if use_qkv_bias:
                            nc.tensor.matmul(
                                pv[0:tsz, :],
                                lhsT=consts[0:1, 256:256 + tsz],  # row of ones
                                rhs=vb[:, half * 512:(half + 1) * 512],
                                start=False, stop=True)
                        nc.vector.tensor_copy(
                            v_s[0:tsz, half * 8:(half + 1) * 8, 0:64], pv[0:tsz, :])
                    nc.vector.memset(v_s[:, :, 64:65], 1.0)
                    vts.append(v_s)
                vt[b] = vts

            def emit_rot2(fp, umul2, tmul2, qk):
                # deferred rope for feature-tile pair fp: pair-swap matmuls + add
                prot2 = sa2("prot2")
                for j in range(2):
                    nc.tensor.matmul(prot2[:, j, 0:NP], lhsT=PI,
                                     rhs=umul2[:, j, :],
                                     start=True, stop=not use_qkv_bias)
                    if use_qkv_bias:
                        nc.tensor.matmul(prot2[:, j, 0:NP], lhsT=I128,
                                         rhs=qb[:, 2 * fp + j, :],
                                         start=False, stop=True)
                nc.vector.tensor_tensor(qk[:, 2 * fp:2 * fp + 2, :],
                                        prot2[:, :, 0:NP], tmul2[:],
                                        mybir.AluOpType.add)

            def emit_qk(b):
                boff = b * NP
                qk = qkpool.tile([128, 16, NP], bf16, name="qk")
                pend = None
                # q/k pairs interleaved so head-pair p's q AND k tiles complete
                # early, in p order — scores never wait on the rope tail
                for fp in (0, 4, 1, 5, 2, 6, 3, 7):
                    pq2 = sa2("pq2")
                    for j in range(2):
                        f = 2 * fp + j
                        if FP8_QK:
                            for c4 in range(4):
                                nc.tensor.matmul(
                                    pq2[:, j, 0:NP],
                                    lhsT=wqk[:, 2 * c4:2 * c4 + 2, f * 128:(f + 1) * 128],
                                    rhs=xt8[:, 2 * c4:2 * c4 + 2, boff: boff + NP],
                                    start=(c4 == 0), stop=(c4 == 3),
                                    perf_mode=DR)
                        else:
                            for c in range(8):
                                nc.tensor.matmul(
                                    pq2[:, j, 0:NP],
                                    lhsT=wqk[:, c, f * 128:(f + 1) * 128],
                                    rhs=xt[:, c, boff: boff + NP],
                                    start=(c == 0), stop=(c == 7))
                    if pend is not None:
                        emit_rot2(*pend, qk)
                    tb = 0 if fp < 4 else 4
                    cos2 = tabs[:, tb:tb + 2, :]
                    sin2 = tabs[:, tb + 2:tb + 4, :]
                    qsb2 = ropool.tile([128, 2, NP], bf16, tag="qs", name="qsb2")
                    nc.scalar.copy(qsb2[:], pq2[:, :, 0:NP])
                    tmul2 = ropool.tile([128, 2, NP], bf16, tag="tm", name="tmul2")
                    nc.vector.tensor_tensor(tmul2[:], qsb2[:], cos2,
                                            mybir.AluOpType.mult)
                    umul2 = ropool.tile([128, 2, NP], bf16, tag="um", name="umul2")
                    nc.vector.tensor_tensor(umul2[:], qsb2[:], sin2,
                                            mybir.AluOpType.mult)
                    pend = (fp, umul2, tmul2)
                emit_rot2(*pend, qk)
                return qk

            def emit_scores(qk, p):
                # hh pairs write the two halves of one PSUM pair tile; the
                # K=64 row-tiled matmuls run concurrently on the PE
                def mm(out, msel, nsel, hh):
                    r0 = hh * 64
                    nc.tensor.matmul(
                        out,
                        lhsT=qk[r0:r0 + 64, 8 + p, msel[0]:msel[1]],
                        rhs=qk[r0:r0 + 64, p, nsel[0]:nsel[1]],
                        start=True, stop=True, tile_position=(r0, 0))
                s0p = sa2("s0p")
                for hh in range(2):
                    mm(s0p[0:128, hh, 0:NP], (0, 128), (0, NP), hh)
                if bc:
                    s12p = sa2("s12p")
                    for hh in range(2):
                        mm(s12p[0:128, hh, 0:NW], (128, 256), (85, 341), hh)
                    for hh in range(2):
                        mm(s12p[0:85, hh, NW:2 * NW], (256, 341), (85, 341), hh)
                    return (s0p, s12p)
                else:
                    s1p = sa2("s1p")
                    for hh in range(2):
                        mm(s1p[0:128, hh, 0:NP], (128, 256), (0, NP), hh)
                    s2p = sa2("s2p")
                    for hh in range(2):
                        mm(s2p[0:85, hh, 0:NP], (256, 341), (0, NP), hh)
                    return (s0p, s1p, s2p)

            def emit_exp(slots, ex2):
                if bc:
                    s0p, s12p = slots
                    nc.scalar.activation(ex2[0:128, :, 0:NP],
                                         s0p[0:128, :, 0:NP], AF.Exp)
                    nc.vector.tensor_tensor(
                        ex2[0:128, :, 0:85], ex2[0:128, :, 0:85],
                        bcm2.rearrange("p (two c) -> p two c", two=2)[:, :, 0:85],
                        mybir.AluOpType.mult)
                    nc.scalar.activation(ex2[0:128, :, NP:NP + 512],
                                         s12p[0:128, :, 0:512], AF.Exp)
                else:
                    for si, (m0, msz) in enumerate(MSL):
                        for hh in range(2):
                            o = EXOFF[si]
                            if general:
                                exr = ropool.tile([128, NP], bf16, tag=f"exr{hh}",
                                                  name="exr")
                                nc.scalar.activation(exr[0:msz, :],
                                                     slots[si][0:msz, hh, 0:NP],
                                                     AF.Exp)
                                nc.vector.tensor_tensor(
                                    ex2[0:msz, hh, o:o + NP],
                                    exr[0:msz, :], maskm[0:msz, si, :],
                                    mybir.AluOpType.mult)
                            else:
                                nc.scalar.activation(
                                    ex2[0:msz, hh, o:o + NP],
                                    slots[si][0:msz, hh, 0:NP], AF.Exp)

            def emit_av(b, p, ex2):
                av2 = avp.tile([128, 2, 512], f32, tag="av", name="av2")
                for hh in range(2):
                    h = 2 * p + hh
                    if bc:
                        nc.tensor.matmul(
                            av2[0:65, hh, 0:NP], lhsT=vt[b][0][0:128, h, :],
                            rhs=ex2[0:128, hh, 0:NP], start=True, stop=False)
                        nc.tensor.matmul(
                            av2[0:65, hh, 85:341], lhsT=vt[b][1][0:128, h, :],
                            rhs=ex2[0:128, hh, NP:NP + NW], start=False, stop=False)
                        nc.tensor.matmul(
                            av2[0:65, hh, 85:341], lhsT=vt[b][2][0:85, h, :],
                            rhs=ex2[0:85, hh, NP + NW:NP + 2 * NW],
                            start=False, stop=True)
                    else:
                        for si, (m0, msz) in enumerate(MSL):
                            nc.tensor.matmul(
                                av2[0:65, hh, 0:NP], lhsT=vt[b][si][0:msz, h, :],
                                rhs=ex2[0:msz, hh, EXOFF[si]:EXOFF[si] + NP],
                                start=(si == 0), stop=(si == 2))
                nc.vector.tensor_copy(asba[b][0:65, 2 * p:2 * p + 2, :],
                                      av2[0:65, :, 0:NP])

            def emit_sc(b, qk, finb):
                asba[b] = asbp.tile([128, 16, NP], bf16, name="asb_all")
                if finb is not None:
                    att[finb] = atpool.tile([128, 8, NP], bf16, name="att")
                pend = None
                for p in range(8):
                    ex2 = expool.tile([128, 2, EXW], bf16, tag="e", name="ex2")
                    slots = emit_scores(qk, p)
                    emit_exp(slots, ex2)
                    if finb is not None:
                        for hh in range(2):
                            nc.vector.tensor_tensor(
                                att[finb][hh * 64:(hh + 1) * 64, p, :],
                                asba[finb][0:64, 2 * p + hh, :],
                                prb[finb][0:64, 2 * p + hh, :],
                                mybir.AluOpType.mult)
                    if pend is not None:
                        emit_av(b, *pend)
                    pend = (p, ex2)
                emit_av(b, *pend)

            def emit_rec(b):
                staged = stpool.tile([16, NP], bf16, tag="staged", name="staged")
                nc.sync.dma_start(staged[:], asba[b][64:65, :, :])
                rec = stpool.tile([16, NP], bf16, tag="rec", name="rec")
                nc.vector.reciprocal(rec[:], staged[:])
                # bounce through DRAM, then broadcast each head's row across
                # 64 partitions with step-0 DMAs
                rd = drp.tile([16, NP], bf16, tag="rd", name="rd")
                nc.sync.dma_start(rd[:], rec[:])
                prb[b] = prbp.tile([64, 16, NP], bf16, name="prb")
                for h in range(16):
                    nc.sync.dma_start(prb[b][0:64, h, :],
                                      rd[h:h + 1, :].to_broadcast([64, NP]))

            def emit_fin(b):
                # standalone normalization (last batch)
                for p in range(8):
                    for hh in range(2):
                        nc.vector.tensor_tensor(
                            att[b][hh * 64:(hh + 1) * 64, p, :],
                            asba[b][0:64, 2 * p + hh, :],
                            prb[b][0:64, 2 * p + hh, :],
                            mybir.AluOpType.mult)

            def emit_proj(b):
                for s, (t0, tsz) in enumerate(MSL):
                    rsz = min(tsz, NTOK - t0)
                    ysb = yspool.tile([128, 1024], bf16, name="ysb")
                    for half in range(2):
                        py = hfp.tile([128, 512], f32, tag="hf", name="py")
                        for c in range(8):
                            nc.tensor.matmul(
                                py[0:tsz, :],
                                lhsT=att[b][:, c, t0:t0 + tsz],
                                rhs=wp[:, c, half * 512:(half + 1) * 512],
                                start=(c == 0), stop=(c == 7))
                        nc.scalar.copy(ysb[0:tsz, half * 512:(half + 1) * 512],
                                       py[0:tsz, :])
                    nc.sync.dma_start(
                        out_d[b * NTOK + t0: b * NTOK + t0 + rsz, :],
                        ysb[0:rsz, :])

            # ---- schedule ----
            emit_v(0); emit_v(1); emit_v(2)
            qk0 = emit_qk(0)
            emit_sc(0, qk0, None); emit_rec(0)
            emit_v(3)
            qk1 = emit_qk(1)
            emit_sc(1, qk1, 0); emit_rec(1)
            emit_proj(0)
            qk2 = emit_qk(2)
            emit_sc(2, qk2, 1); emit_rec(2)
            emit_proj(1)
            qk3 = emit_qk(3)
            emit_sc(3, qk3, 2); emit_rec(3)
            emit_proj(2)
            att[3] = atpool.tile([128, 8, NP], bf16, name="att3")
            emit_fin(3)
            emit_proj(3)
    nc.finalize()
    return nc


def _get_nc(mask_mode, use_qkv_bias):
    key = (mask_mode, use_qkv_bias)
    if key not in _cache:
        _cache[key] = _build(mask_mode, use_qkv_bias)
    return _cache[key]


def _bc_mask():
    seg = np.concatenate([np.full(s * s, i, dtype=np.int64) for i, s in enumerate(SCALES)])
    allow = seg[:, None] >= seg[None, :]
    return np.where(allow, 0.0, -1e9).astype(np.float32)[None, None]


def _prep_core_inputs(x, mask, qkv_w, qkv_b, proj_w, proj_b):
    tabs, consts = _host_tables()
    mf = mask.astype(np.float32)
    if not np.any(mf != 0):
        mask_mode = "none"
    elif np.array_equal(mf, _bc_mask()):
        mask_mode = "bc"
    else:
        mask_mode = "general"
    use_qb = bool(np.any(qkv_b != 0))

    wqkT = qkv_w.astype(np.float32).T                    # [1024, 3072]
    wv = np.ascontiguousarray(wqkT[:, 2048:].astype(BF16))
    wpT = np.ascontiguousarray(proj_w.astype(np.float32).T.astype(BF16))

    common = {"wv": wv, "wp": wpT, "tabs": np.ascontiguousarray(tabs),
              "consts": np.ascontiguousarray(consts)}
    if FP8_QK:
        common["wqk8"] = np.ascontiguousarray(
            (wqkT[:, :2048] * W8SCALE).astype(F8))
    else:
        common["wqk"] = np.ascontiguousarray(wqkT[:, :2048].astype(BF16))
    if mask_mode == "general":
        mT = mask[0, 0].astype(np.float32).T            # [keys, queries]
        mm = np.zeros((384, NP), np.float32)
        mm[:NTOK, :NTOK] = np.exp(mT)                   # multiplicative mask
        maskm = np.zeros((128, 3 * NP), np.float32)
        for s in range(3):
            maskm[:, s * NP:(s + 1) * NP] = mm[s * 128:(s + 1) * 128, :]
        common["maskm"] = maskm.astype(BF16)
    if use_qb:
        cos, sin = _rope_tables()
        sin2 = sin.copy(); sin2[:, 0::2] = -sin[:, 0::2]
        scale = 1.0 / np.sqrt(HD)
        qb_full = np.zeros((128, 16 * NP), np.float32)
        bq = qkv_b[:2048].astype(np.float32)
        for f in range(16):
            is_q = f < 8
            sc = scale if is_q else 1.0
            for hh in range(2):
                hvec = bq[f * 128 + hh * 64: f * 128 + (hh + 1) * 64]  # [64]
                hswap = hvec.reshape(-1, 2)[:, ::-1].reshape(-1)
                rb = cos * hvec[None, :] + sin2 * hswap[None, :]       # [341,64]
                qb_full[hh * 64:(hh + 1) * 64, f * NP: f * NP + NTOK] = sc * rb.T
        common["qb"] = qb_full.astype(BF16)
        common["vb"] = qkv_b[2048:].astype(np.float32).astype(BF16)[None, :]

    in_maps = []
    xf = x.astype(np.float32)
    for core in range(NCORES):
        xc = xf[core * BPC:(core + 1) * BPC]            # [4, 341, 1024]
        xp = np.zeros((BPC, NP, DIM), np.float32)
        xp[:, :NTOK, :] = xc
        xT = np.ascontiguousarray(xp.reshape(BPC * NP, DIM).T)  # [1024, 1376]
        m = dict(common)
        m["xt"] = xT.astype(BF16)
        if FP8_QK:
            m["xt8"] = xT.astype(F8)
        in_maps.append(m)
    return in_maps, mask_mode, use_qb


def kernel(x, mask, qkv_w, qkv_b, proj_w, proj_b, _trace=False):
    from concourse.bass_utils import run_bass_kernel_spmd
    x, mask, qkv_w, qkv_b, proj_w, proj_b = (
        np.asarray(t) for t in (x, mask, qkv_w, qkv_b, proj_w, proj_b))
    in_maps, mask_mode, use_qb = _prep_core_inputs(
        x, mask, qkv_w, qkv_b, proj_w, proj_b)
    nc = _get_nc(mask_mode, use_qb)
    res = run_bass_kernel_spmd(nc, in_maps, core_ids=list(range(NCORES)),
                               trace=_trace)
    out = np.empty((B, NTOK, DIM), np.float32)
    for core in range(NCORES):
        y = res.results[core]["out"].astype(np.float32).reshape(BPC, NTOK, DIM)
        out[core * BPC:(core + 1) * BPC] = y
    pb = proj_b.astype(np.float32)
    if np.any(pb != 0):
        out += pb[None, None, :]
    kernel._last_exec_time_ns = res.exec_time_ns
    kernel._last_results = res
    return out


# revision 36
# speedup vs baseline: 1.0405x; 1.0405x over previous
"""Trainium2 Bass kernel for a 16-head attention block with 2D axial RoPE.

Strategy: pure data-parallel over batch (32 batches -> 4 per NeuronCore),
bf16 compute with an fp8 (DoubleRow) QKV q/k projection, feature-major
("transposed") layouts throughout:
  - q/k stay feature-major from the QKV projection (fp8 weights x256 with
    the 1/256 folded into the rope tables; errors attenuate through
    softmax); v is produced token-major directly in bf16.
  - RoPE via two elementwise muls + a pair-swap permutation matmul;
    feature tiles processed in pairs sharing [128,2,512] PSUM tiles so
    every elementwise op covers two tiles in one instruction.
  - scoresT[m,n] per head (keys on partitions): row-packed K=64 matmul
    pairs via tile_position writing the two halves of one PSUM pair
    tile; si1+si2 share one bank; exp runs as two activations per head
    pair; block-causal mask applied as one small 0/1-mask multiply over
    query columns 0:85; softmax without max subtraction; per-query sums
    via an appended ones-column on v; normalization via DMA-broadcast
    reciprocal sums + one bf16 multiply per head.
  - proj maps back to token-major; bf16 output DMA.
"""
import sys, os
sys.path.insert(0, "/opt/trn_rl_repo")
import numpy as np
import ml_dtypes

B, NTOK, DIM, H, HD = 32, 341, 1024, 16, 64
NCORES, BPC = 8, 4          # cores, batches per core
NP = 344                    # padded tokens per batch (bf16 pair aligned)
T = BPC * NP                # 1376 tokens per core
SCALES = [1, 2, 4, 8, 16]
PT_SEQ_LEN, THETA = 16, 10000.0
ROPE_DIM = HD // 2
MSL = [(0, 128), (128, 128), (256, 85)]   # m/token slices per batch
NW = 341 - 85               # si>0 query window size (= 256, one half-bank)
FP8_QK = False              # fp8 DoubleRow q/k projection (fails 2e-2 gate)
W8SCALE = 256.0
BF16 = ml_dtypes.bfloat16
F8 = ml_dtypes.float8_e4m3fn

_cache = {}


def _rope_tables():
    inv = 1.0 / (THETA ** (np.arange(0, ROPE_DIM, 2, dtype=np.float64) / ROPE_DIM))
    cos_list, sin_list = [], []
    for s in SCALES:
        t = np.arange(s, dtype=np.float64) / s * PT_SEQ_LEN
        f = np.outer(t, inv)
        f = np.repeat(f, 2, axis=-1)
        fy = np.broadcast_to(f[:, None, :], (s, s, ROPE_DIM))
        fx = np.broadcast_to(f[None, :, :], (s, s, ROPE_DIM))
        ff = np.concatenate([fy, fx], axis=-1).reshape(s * s, HD)
        cos_list.append(np.cos(ff))
        sin_list.append(np.sin(ff))
    cos = np.concatenate(cos_list, axis=0).astype(np.float32)  # [341, 64]
    sin = np.concatenate(sin_list, axis=0).astype(np.float32)
    return cos, sin


def _host_tables():
    cos, sin = _rope_tables()               # [341, 64]
    # sin2: sign pattern for rotate_half: q'[2i] = q[2i]c - q[2i+1]s ...
    sin2 = sin.copy()
    sin2[:, 0::2] = -sin[:, 0::2]
    # sinP[e] = sin2[e^1] (so that (PI @ (q*sinP))[d] = q[d^1]*sin2[d])
    sinP = np.empty_like(sin2)
    sinP[:, 0::2] = sin2[:, 1::2]
    sinP[:, 1::2] = sin2[:, 0::2]
    cosT = np.zeros((HD, NP), np.float32)
    sinPT = np.zeros((HD, NP), np.float32)
    cosT[:, :NTOK] = cos.T
    sinPT[:, :NTOK] = sinP.T
    cos128 = np.vstack([cosT, cosT])        # [128, NP] two heads per tile
    sinP128 = np.vstack([sinPT, sinPT])
    qs = (1.0 / np.sqrt(HD)) / (W8SCALE if FP8_QK else 1.0)
    ks = 1.0 / (W8SCALE if FP8_QK else 1.0)
    # tabs: [cosq, cosq, sinq, sinq, cosk, cosk, sink, sink] (f-pair duplicated)
    blocks = [cos128 * qs, cos128 * qs, sinP128 * qs, sinP128 * qs,
              cos128 * ks, cos128 * ks, sinP128 * ks, sinP128 * ks]
    tabs = np.concatenate(blocks, axis=1)   # [128, 8*NP]
    # consts: PI | I128 | ones | bcm2 (duplicated per head-pair)
    PI = np.zeros((128, 128), np.float32)
    for d in range(128):
        PI[d ^ 1, d] = 1.0
    I128 = np.eye(128, dtype=np.float32)
    ones = np.ones((128, 128), dtype=np.float32)
    seg = np.concatenate([np.full(s * s, i, dtype=np.int64) for i, s in enumerate(SCALES)])
    bcm = np.ones((128, 96), dtype=np.float32)
    bcm[:, 0:85] = (seg[:128, None] <= seg[None, :85]).astype(np.float32)
    consts = np.concatenate([PI, I128, ones, bcm, bcm], axis=1)  # [128, 576]
    return tabs.astype(BF16), consts.astype(BF16)


def _build(mask_mode, use_qkv_bias):
    import concourse.bass as bass
    import concourse.bacc as bacc
    import concourse.tile as tile
    from concourse import mybir

    f32, bf16 = mybir.dt.float32, mybir.dt.bfloat16
    f8 = mybir.dt.float8e4
    AF = mybir.ActivationFunctionType
    DR = mybir.MatmulPerfMode.DoubleRow
    nc = bacc.Bacc("TRN2", target_bir_lowering=False, debug=False)

    xt_d = nc.dram_tensor("xt", [DIM, T], bf16, kind="ExternalInput")
    if FP8_QK:
        xt8_d = nc.dram_tensor("xt8", [DIM, T], f8, kind="ExternalInput")
        wqk8_d = nc.dram_tensor("wqk8", [DIM, 2048], f8, kind="ExternalInput")
    else:
        wqk_d = nc.dram_tensor("wqk", [DIM, 2048], bf16, kind="ExternalInput")
    wv_d = nc.dram_tensor("wv", [DIM, 1024], bf16, kind="ExternalInput")
    wp_d = nc.dram_tensor("wp", [DIM, 1024], bf16, kind="ExternalInput")
    tabs_d = nc.dram_tensor("tabs", [128, 8 * NP], bf16, kind="ExternalInput")
    consts_d = nc.dram_tensor("consts", [128, 576], bf16, kind="ExternalInput")
    general = mask_mode == "general"
    if general:
        maskm_d = nc.dram_tensor("maskm", [128, 3 * NP], bf16, kind="ExternalInput")
    if use_qkv_bias:
        qb_d = nc.dram_tensor("qb", [128, 16 * NP], bf16, kind="ExternalInput")
        vb_d = nc.dram_tensor("vb", [1, 1024], bf16, kind="ExternalInput")
    out_d = nc.dram_tensor("out", [BPC * NTOK, DIM], bf16, kind="ExternalOutput")

    bc = mask_mode == "bc"
    # ex free-dim layout: bc packs si1+si2 as one 512-col block
    EXW = NP + 2 * 256 if bc else 3 * NP
    EXOFF = [0, NP, NP + 256] if bc else [0, NP, 2 * NP]

    with tile.TileContext(nc) as tc, \
         nc.allow_low_precision(reason="bf16/fp8 qk; rel gate 2e-2"):
        with tc.tile_pool(name="res", bufs=1) as res, \
             tc.tile_pool(name="vp", bufs=9) as vpool, \
             tc.tile_pool(name="qkp", bufs=2) as qkpool, \
             tc.tile_pool(name="ro", bufs=2) as ropool, \
             tc.tile_pool(name="ex", bufs=2) as expool, \
             tc.tile_pool(name="asb", bufs=2) as asbp, \
             tc.tile_pool(name="st", bufs=1) as stpool, \
             tc.tile_pool(name="at", bufs=2) as atpool, \
             tc.tile_pool(name="prb", bufs=1) as prbp, \
             tc.tile_pool(name="ys", bufs=2) as yspool, \
             tc.tile_pool(name="dr", bufs=1, space="DRAM") as drp, \
             tc.tile_pool(name="sa", bufs=2, space="PSUM") as sap, \
             tc.tile_pool(name="av", bufs=1, space="PSUM") as avp, \
             tc.tile_pool(name="hf", bufs=2, space="PSUM") as hfp:

            # ---- resident loads (order matters: v(0..2) cover the rest) ----
            xt = res.tile([128, 8, T], bf16)
            if FP8_QK:
                xt8 = res.tile([128, 8, T], f8)
                wqk = res.tile([128, 8, 2048], f8)
            else:
                wqk = res.tile([128, 8, 2048], bf16)
            wv = res.tile([128, 8, 1024], bf16)
            wp = res.tile([128, 8, 1024], bf16)
            for c in range(8):
                nc.sync.dma_start(wv[:, c, :], wv_d[c * 128:(c + 1) * 128, :])
                nc.sync.dma_start(xt[:, c, 0:NP], xt_d[c * 128:(c + 1) * 128, 0:NP])
            for b in range(1, 3):
                for c in range(8):
                    nc.sync.dma_start(xt[:, c, b * NP:(b + 1) * NP],
                                      xt_d[c * 128:(c + 1) * 128, b * NP:(b + 1) * NP])
            tabs = res.tile([128, 8, NP], bf16)
            nc.sync.dma_start(tabs[:], tabs_d[:])
            consts = res.tile([128, 576], bf16)
            nc.sync.dma_start(consts[:], consts_d[:])
            for c in range(8):
                if FP8_QK:
                    nc.sync.dma_start(wqk[:, c, :], wqk8_d[c * 128:(c + 1) * 128, :])
                    nc.sync.dma_start(xt8[:, c, :], xt8_d[c * 128:(c + 1) * 128, :])
                else:
                    nc.sync.dma_start(wqk[:, c, :], wqk_d[c * 128:(c + 1) * 128, :])
            for c in range(8):
                nc.sync.dma_start(wp[:, c, :], wp_d[c * 128:(c + 1) * 128, :])
            for c in range(8):
                nc.sync.dma_start(xt[:, c, 3 * NP:T],
                                  xt_d[c * 128:(c + 1) * 128, 3 * NP:T])
            if general:
                maskm = res.tile([128, 3, NP], bf16)
                nc.sync.dma_start(maskm[:], maskm_d[:])
            if use_qkv_bias:
                qb = res.tile([128, 16, NP], bf16)
                nc.sync.dma_start(qb[:], qb_d[:])
                vb = res.tile([1, 1024], bf16)
                nc.sync.dma_start(vb[:], vb_d[:])

            PI = consts[:, 0:128]
            I128 = consts[:, 128:256]
            bcm2 = consts[:, 384:576]  # [128, 2, 96] view below

            def sa2(name):
                return sap.tile([128, 2, 512], f32, tag="sa", name=name)

            vt = {}     # b -> [v_s tiles per slice]
            att = {}    # b -> att tile
            prb = {}    # b -> broadcast reciprocal sums [64, 16, NP]
            asba = {}   # b -> asb_all [128, 16, NP]

            def emit_v(b):
                boff = b * NP
                vts = []
                for s, (t0, tsz) in enumerate(MSL):
                    v_s = vpool.tile([128, 16, 65], bf16, name="v_s")
                    for half in range(2):
                        pv = hfp.tile([128, 512], f32, tag="hf", name="pv")
                        for c in range(8):
                            nc.tensor.matmul(
                                pv[0:tsz, :],
                                lhsT=xt[:, c, boff + t0: boff + t0 + tsz],
                                rhs=wv[:, c, half * 512:(half + 1) * 512],
                                start=(c == 0), stop=(c == 7 and not use_qkv_bias))
                        if use_qkv_bias:
                            nc.tensor.matmul(
                                pv[0:tsz, :],
                                lhsT=consts[0:1, 256:256 + tsz],  # row of ones
                                rhs=vb[:, half * 512:(half + 1) * 512],
                                start=False, stop=True)
                        nc.vector.tensor_copy(
                            v_s[0:tsz, half * 8:(half + 1) * 8, 0:64], pv[0:tsz, :])
                    nc.vector.memset(v_s[:, :, 64:65], 1.0)
                    vts.append(v_s)
                vt[b] = vts

            def emit_rot2(fp, umul2, tmul2, qk):
                # deferred rope for feature-tile pair fp: pair-swap matmuls + add
                prot2 = sa2("prot2")
                for j in range(2):
                    nc.tensor.matmul(prot2[:, j, 0:NP], lhsT=PI,
                                     rhs=umul2[:, j, :],
                                     start=True, stop=not use_qkv_bias)
                    if use_qkv_bias:
                        nc.tensor.matmul(prot2[:, j, 0:NP], lhsT=I128,
                                         rhs=qb[:, 2 * fp + j, :],
                                         start=False, stop=True)
                nc.vector.tensor_tensor(qk[:, 2 * fp:2 * fp + 2, :],
                                        prot2[:, :, 0:NP], tmul2[:],
                                        mybir.AluOpType.add)

            def emit_qk(b):
                boff = b * NP
                qk = qkpool.tile([128, 16, NP], bf16, name="qk")
                pend = None
                # q/k pairs interleaved so head-pair p's q AND k tiles complete
                # early, in p order — scores never wait on the rope tail
                for fp in (0, 4, 1, 5, 2, 6, 3, 7):
                    pq2 = sa2("pq2")
                    qsb2 = None
                    for j in range(2):
                        f = 2 * fp + j
                        if FP8_QK:
                            for c4 in range(4):
                                nc.tensor.matmul(
                                    pq2[:, j, 0:NP],
                                    lhsT=wqk[:, 2 * c4:2 * c4 + 2, f * 128:(f + 1) * 128],
                                    rhs=xt8[:, 2 * c4:2 * c4 + 2, boff: boff + NP],
                                    start=(c4 == 0), stop=(c4 == 3),
                                    perf_mode=DR)
                        else:
                            for c in range(8):
                                nc.tensor.matmul(
                                    pq2[:, j, 0:NP],
                                    lhsT=wqk[:, c, f * 128:(f + 1) * 128],
                                    rhs=xt[:, c, boff: boff + NP],
                                    start=(c == 0), stop=(c == 7))
                        if j == 0:
                            # copy half 0 while half 1's matmuls stream, so the
                            # pair slot frees right after half 1 finishes
                            qsb2 = ropool.tile([128, 2, NP], bf16, tag="qs",
                                               name="qsb2")
                            nc.scalar.copy(qsb2[:, 0, :], pq2[:, 0, 0:NP])
                    nc.scalar.copy(qsb2[:, 1, :], pq2[:, 1, 0:NP])
                    if pend is not None:
                        emit_rot2(*pend, qk)
                    tb = 0 if fp < 4 else 4
                    cos2 = tabs[:, tb:tb + 2, :]
                    sin2 = tabs[:, tb + 2:tb + 4, :]
                    tmul2 = ropool.tile([128, 2, NP], bf16, tag="tm", name="tmul2")
                    nc.vector.tensor_tensor(tmul2[:], qsb2[:], cos2,
                                            mybir.AluOpType.mult)
                    umul2 = ropool.tile([128, 2, NP], bf16, tag="um", name="umul2")
                    nc.vector.tensor_tensor(umul2[:], qsb2[:], sin2,
                                            mybir.AluOpType.mult)
                    pend = (fp, umul2, tmul2)
                emit_rot2(*pend, qk)
                return qk

            def emit_scores(qk, p):
                # hh pairs write the two halves of one PSUM pair tile; the
                # K=64 row-tiled matmuls run concurrently on the PE
                def mm(out, msel, nsel, hh):
                    r0 = hh * 64
                    nc.tensor.matmul(
                        out,
                        lhsT=qk[r0:r0 + 64, 8 + p, msel[0]:msel[1]],
                        rhs=qk[r0:r0 + 64, p, nsel[0]:nsel[1]],
                        start=True, stop=True, tile_position=(r0, 0))
                s0p = sa2("s0p")
                for hh in range(2):
                    mm(s0p[0:128, hh, 0:NP], (0, 128), (0, NP), hh)
                if bc:
                    s12p = sa2("s12p")
                    for hh in range(2):
                        mm(s12p[0:128, hh, 0:NW], (128, 256), (85, 341), hh)
                    for hh in range(2):
                        mm(s12p[0:85, hh, NW:2 * NW], (256, 341), (85, 341), hh)
                    return (s0p, s12p)
                else:
                    s1p = sa2("s1p")
                    for hh in range(2):
                        mm(s1p[0:128, hh, 0:NP], (128, 256), (0, NP), hh)
                    s2p = sa2("s2p")
                    for hh in range(2):
                        mm(s2p[0:85, hh, 0:NP], (256, 341), (0, NP), hh)
                    return (s0p, s1p, s2p)

            def emit_exp(slots, ex2):
                if bc:
                    s0p, s12p = slots
                    nc.scalar.activation(ex2[0:128, :, 0:NP],
                                         s0p[0:128, :, 0:NP], AF.Exp)
                    nc.vector.tensor_tensor(
                        ex2[0:128, :, 0:85], ex2[0:128, :, 0:85],
                        bcm2.rearrange("p (two c) -> p two c", two=2)[:, :, 0:85],
                        mybir.AluOpType.mult)
                    nc.scalar.activation(ex2[0:128, :, NP:NP + 512],
                                         s12p[0:128, :, 0:512], AF.Exp)
                else:
                    for si, (m0, msz) in enumerate(MSL):
                        for hh in range(2):
                            o = EXOFF[si]
                            if general:
                                exr = ropool.tile([128, NP], bf16, tag=f"exr{hh}",
                                                  name="exr")
                                nc.scalar.activation(exr[0:msz, :],
                                                     slots[si][0:msz, hh, 0:NP],
                                                     AF.Exp)
                                nc.vector.tensor_tensor(
                                    ex2[0:msz, hh, o:o + NP],
                                    exr[0:msz, :], maskm[0:msz, si, :],
                                    mybir.AluOpType.mult)
                            else:
                                nc.scalar.activation(
                                    ex2[0:msz, hh, o:o + NP],
                                    slots[si][0:msz, hh, 0:NP], AF.Exp)

            def emit_av(b, p, ex2):
                av2 = avp.tile([128, 2, 512], f32, tag="av", name="av2")
                for hh in range(2):
                    h = 2 * p + hh
                    if bc:
                        nc.tensor.matmul(
                            av2[0:65, hh, 0:NP], lhsT=vt[b][0][0:128, h, :],
                            rhs=ex2[0:128, hh, 0:NP], start=True, stop=False)
                        nc.tensor.matmul(
                            av2[0:65, hh, 85:341], lhsT=vt[b][1][0:128, h, :],
                            rhs=ex2[0:128, hh, NP:NP + NW], start=False, stop=False)
                        nc.tensor.matmul(
                            av2[0:65, hh, 85:341], lhsT=vt[b][2][0:85, h, :],
                            rhs=ex2[0:85, hh, NP + NW:NP + 2 * NW],
                            start=False, stop=True)
                    else:
                        for si, (m0, msz) in enumerate(MSL):
                            nc.tensor.matmul(
                                av2[0:65, hh, 0:NP], lhsT=vt[b][si][0:msz, h, :],
                                rhs=ex2[0:msz, hh, EXOFF[si]:EXOFF[si] + NP],
                                start=(si == 0), stop=(si == 2))
                nc.vector.tensor_copy(asba[b][0:65, 2 * p:2 * p + 2, :],
                                      av2[0:65, :, 0:NP])

            def emit_sc(b, qk, finb):
                asba[b] = asbp.tile([128, 16, NP], bf16, name="asb_all")
                if finb is not None:
                    att[finb] = atpool.tile([128, 8, NP], bf16, name="att")
                pend = None
                for p in range(8):
                    ex2 = expool.tile([128, 2, EXW], bf16, tag="e", name="ex2")
                    slots = emit_scores(qk, p)
                    emit_exp(slots, ex2)
                    if finb is not None:
                        for hh in range(2):
                            nc.vector.tensor_tensor(
                                att[finb][hh * 64:(hh + 1) * 64, p, :],
                                asba[finb][0:64, 2 * p + hh, :],
                                prb[finb][0:64, 2 * p + hh, :],
                                mybir.AluOpType.mult)
                    if pend is not None:
                        emit_av(b, *pend)
                    pend = (p, ex2)
                emit_av(b, *pend)

            def emit_rec(b):
                staged = stpool.tile([16, NP], bf16, tag="staged", name="staged")
                nc.sync.dma_start(staged[:], asba[b][64:65, :, :])
                rec = stpool.tile([16, NP], bf16, tag="rec", name="rec")
                nc.vector.reciprocal(rec[:], staged[:])
                # bounce through DRAM, then broadcast each head's row across
                # 64 partitions with step-0 DMAs
                rd = drp.tile([16, NP], bf16, tag="rd", name="rd")
                nc.sync.dma_start(rd[:], rec[:])
                prb[b] = prbp.tile([64, 16, NP], bf16, name="prb")
                for h in range(16):
                    nc.sync.dma_start(prb[b][0:64, h, :],
                                      rd[h:h + 1, :].to_broadcast([64, NP]))

            def emit_fin(b):
                # standalone normalization (last batch)
                for p in range(8):
                    for hh in range(2):
                        nc.vector.tensor_tensor(
                            att[b][hh * 64:(hh + 1) * 64, p, :],
                            asba[b][0:64, 2 * p + hh, :],
                            prb[b][0:64, 2 * p + hh, :],
                            mybir.AluOpType.mult)

            def emit_proj(b):
                for s, (t0, tsz) in enumerate(MSL):
                    rsz = min(tsz, NTOK - t0)
                    ysb = yspool.tile([128, 1024], bf16, name="ysb")
                    for half in range(2):
                        py = hfp.tile([128, 512], f32, tag="hf", name="py")
                        for c in range(8):
                            nc.tensor.matmul(
                                py[0:tsz, :],
                                lhsT=att[b][:, c, t0:t0 + tsz],
                                rhs=wp[:, c, half * 512:(half + 1) * 512],
                                start=(c == 0), stop=(c == 7))
                        nc.scalar.copy(ysb[0:tsz, half * 512:(half + 1) * 512],
                                       py[0:tsz, :])
                    nc.sync.dma_start(
                        out_d[b * NTOK + t0: b * NTOK + t0 + rsz, :],
                        ysb[0:rsz, :])

            # ---- schedule ----
            emit_v(0); emit_v(1); emit_v(2)
            qk0 = emit_qk(0)
            emit_sc(0, qk0, None); emit_rec(0)
            emit_v(3)
            qk1 = emit_qk(1)
            emit_sc(1, qk1, 0); emit_rec(1)
            emit_proj(0)
            qk2 = emit_qk(2)
            emit_sc(2, qk2, 1); emit_rec(2)
            emit_proj(1)
            qk3 = emit_qk(3)
            emit_sc(3, qk3, 2); emit_rec(3)
            emit_proj(2)
            att[3] = atpool.tile([128, 8, NP], bf16, name="att3")
            emit_fin(3)
            emit_proj(3)
    nc.finalize()
    return nc


def _get_nc(mask_mode, use_qkv_bias):
    key = (mask_mode, use_qkv_bias)
    if key not in _cache:
        _cache[key] = _build(mask_mode, use_qkv_bias)
    return _cache[key]


def _bc_mask():
    seg = np.concatenate([np.full(s * s, i, dtype=np.int64) for i, s in enumerate(SCALES)])
    allow = seg[:, None] >= seg[None, :]
    return np.where(allow, 0.0, -1e9).astype(np.float32)[None, None]


def _prep_core_inputs(x, mask, qkv_w, qkv_b, proj_w, proj_b):
    tabs, consts = _host_tables()
    mf = mask.astype(np.float32)
    if not np.any(mf != 0):
        mask_mode = "none"
    elif np.array_equal(mf, _bc_mask()):
        mask_mode = "bc"
    else:
        mask_mode = "general"
    use_qb = bool(np.any(qkv_b != 0))

    wqkT = qkv_w.astype(np.float32).T                    # [1024, 3072]
    wv = np.ascontiguousarray(wqkT[:, 2048:].astype(BF16))
    wpT = np.ascontiguousarray(proj_w.astype(np.float32).T.astype(BF16))

    common = {"wv": wv, "wp": wpT, "tabs": np.ascontiguousarray(tabs),
              "consts": np.ascontiguousarray(consts)}
    if FP8_QK:
        common["wqk8"] = np.ascontiguousarray(
            (wqkT[:, :2048] * W8SCALE).astype(F8))
    else:
        common["wqk"] = np.ascontiguousarray(wqkT[:, :2048].astype(BF16))
    if mask_mode == "general":
        mT = mask[0, 0].astype(np.float32).T            # [keys, queries]
        mm = np.zeros((384, NP), np.float32)
        mm[:NTOK, :NTOK] = np.exp(mT)                   # multiplicative mask
        maskm = np.zeros((128, 3 * NP), np.float32)
        for s in range(3):
            maskm[:, s * NP:(s + 1) * NP] = mm[s * 128:(s + 1) * 128, :]
        common["maskm"] = maskm.astype(BF16)
    if use_qb:
        cos, sin = _rope_tables()
        sin2 = sin.copy(); sin2[:, 0::2] = -sin[:, 0::2]
        scale = 1.0 / np.sqrt(HD)
        qb_full = np.zeros((128, 16 * NP), np.float32)
        bq = qkv_b[:2048].astype(np.float32)
        for f in range(16):
            is_q = f < 8
            sc = scale if is_q else 1.0
            for hh in range(2):
                hvec = bq[f * 128 + hh * 64: f * 128 + (hh + 1) * 64]  # [64]
                hswap = hvec.reshape(-1, 2)[:, ::-1].reshape(-1)
                rb = cos * hvec[None, :] + sin2 * hswap[None, :]       # [341,64]
                qb_full[hh * 64:(hh + 1) * 64, f * NP: f * NP + NTOK] = sc * rb.T
        common["qb"] = qb_full.astype(BF16)
        common["vb"] = qkv_b[2048:].astype(np.float32).astype(BF16)[None, :]

    in_maps = []
    xf = x.astype(np.float32)
    for core in range(NCORES):
        xc = xf[core * BPC:(core + 1) * BPC]            # [4, 341, 1024]
        xp = np.zeros((BPC, NP, DIM), np.float32)
        xp[:, :NTOK, :] = xc
        xT = np.ascontiguousarray(xp.reshape(BPC * NP, DIM).T)  # [1024, 1376]
        m = dict(common)
        m["xt"] = xT.astype(BF16)
        if FP8_QK:
            m["xt8"] = xT.astype(F8)
        in_maps.append(m)
    return in_maps, mask_mode, use_qb


def kernel(x, mask, qkv_w, qkv_b, proj_w, proj_b, _trace=False):
    from concourse.bass_utils import run_bass_kernel_spmd
    x, mask, qkv_w, qkv_b, proj_w, proj_b = (
        np.asarray(t) for t in (x, mask, qkv_w, qkv_b, proj_w, proj_b))
    in_maps, mask_mode, use_qb = _prep_core_inputs(
        x, mask, qkv_w, qkv_b, proj_w, proj_b)
    nc = _get_nc(mask_mode, use_qb)
    res = run_bass_kernel_spmd(nc, in_maps, core_ids=list(range(NCORES)),
                               trace=_trace)
    out = np.empty((B, NTOK, DIM), np.float32)
    for core in range(NCORES):
        y = res.results[core]["out"].astype(np.float32).reshape(BPC, NTOK, DIM)
        out[core * BPC:(core + 1) * BPC] = y
    pb = proj_b.astype(np.float32)
    if np.any(pb != 0):
        out += pb[None, None, :]
    kernel._last_exec_time_ns = res.exec_time_ns
    kernel._last_results = res
    return out


# revision 38
# speedup vs baseline: 1.1283x; 1.0844x over previous
"""Trainium2 Bass kernel for a 16-head attention block with 2D axial RoPE.

Strategy: pure data-parallel over batch (32 batches -> 4 per NeuronCore),
bf16 compute with an fp8 (DoubleRow) QKV q/k projection, feature-major
("transposed") layouts throughout:
  - q/k stay feature-major from the QKV projection (fp8 weights x256 with
    the 1/256 folded into the rope tables; errors attenuate through
    softmax); v is produced token-major directly in bf16.
  - RoPE via two elementwise muls + a pair-swap permutation matmul;
    feature tiles processed in pairs sharing [128,2,512] PSUM tiles so
    every elementwise op covers two tiles in one instruction.
  - scoresT[m,n] per head (keys on partitions): row-packed K=64 matmul
    pairs via tile_position writing the two halves of one PSUM pair
    tile; si1+si2 share one bank; exp runs as two activations per head
    pair; block-causal mask applied as one small 0/1-mask multiply over
    query columns 0:85; softmax without max subtraction; per-query sums
    via an appended ones-column on v; normalization via DMA-broadcast
    reciprocal sums + one bf16 multiply per head.
  - proj maps back to token-major; bf16 output DMA.
"""
import sys, os
sys.path.insert(0, "/opt/trn_rl_repo")
import numpy as np
import ml_dtypes

B, NTOK, DIM, H, HD = 32, 341, 1024, 16, 64
NCORES, BPC = 8, 4          # cores, batches per core
NP = 344                    # padded tokens per batch (bf16 pair aligned)
T = BPC * NP                # 1376 tokens per core
SCALES = [1, 2, 4, 8, 16]
PT_SEQ_LEN, THETA = 16, 10000.0
ROPE_DIM = HD // 2
MSL = [(0, 128), (128, 128), (256, 85)]   # m/token slices per batch
NW = 341 - 85               # si>0 query window size (= 256, one half-bank)
FP8_QK = False              # fp8 DoubleRow q/k projection (fails 2e-2 gate)
W8SCALE = 256.0
BF16 = ml_dtypes.bfloat16
F8 = ml_dtypes.float8_e4m3fn

_cache = {}


def _rope_tables():
    inv = 1.0 / (THETA ** (np.arange(0, ROPE_DIM, 2, dtype=np.float64) / ROPE_DIM))
    cos_list, sin_list = [], []
    for s in SCALES:
        t = np.arange(s, dtype=np.float64) / s * PT_SEQ_LEN
        f = np.outer(t, inv)
        f = np.repeat(f, 2, axis=-1)
        fy = np.broadcast_to(f[:, None, :], (s, s, ROPE_DIM))
        fx = np.broadcast_to(f[None, :, :], (s, s, ROPE_DIM))
        ff = np.concatenate([fy, fx], axis=-1).reshape(s * s, HD)
        cos_list.append(np.cos(ff))
        sin_list.append(np.sin(ff))
    cos = np.concatenate(cos_list, axis=0).astype(np.float32)  # [341, 64]
    sin = np.concatenate(sin_list, axis=0).astype(np.float32)
    return cos, sin


def _host_tables():
    cos, sin = _rope_tables()               # [341, 64]
    # sin2: sign pattern for rotate_half: q'[2i] = q[2i]c - q[2i+1]s ...
    sin2 = sin.copy()
    sin2[:, 0::2] = -sin[:, 0::2]
    # sinP[e] = sin2[e^1] (so that (PI @ (q*sinP))[d] = q[d^1]*sin2[d])
    sinP = np.empty_like(sin2)
    sinP[:, 0::2] = sin2[:, 1::2]
    sinP[:, 1::2] = sin2[:, 0::2]
    cosT = np.zeros((HD, NP), np.float32)
    sinPT = np.zeros((HD, NP), np.float32)
    cosT[:, :NTOK] = cos.T
    sinPT[:, :NTOK] = sinP.T
    cos128 = np.vstack([cosT, cosT])        # [128, NP] two heads per tile
    sinP128 = np.vstack([sinPT, sinPT])
    qs = (1.0 / np.sqrt(HD)) / (W8SCALE if FP8_QK else 1.0)
    ks = 1.0 / (W8SCALE if FP8_QK else 1.0)
    # tabs: [cosq, cosq, sinq, sinq, cosk, cosk, sink, sink] (f-pair duplicated)
    blocks = [cos128 * qs, cos128 * qs, sinP128 * qs, sinP128 * qs,
              cos128 * ks, cos128 * ks, sinP128 * ks, sinP128 * ks]
    tabs = np.concatenate(blocks, axis=1)   # [128, 8*NP]
    # consts: PI | I128 | ones | bcm2 (duplicated per head-pair)
    PI = np.zeros((128, 128), np.float32)
    for d in range(128):
        PI[d ^ 1, d] = 1.0
    I128 = np.eye(128, dtype=np.float32)
    ones = np.ones((128, 128), dtype=np.float32)
    seg = np.concatenate([np.full(s * s, i, dtype=np.int64) for i, s in enumerate(SCALES)])
    bcm = np.ones((128, 96), dtype=np.float32)
    bcm[:, 0:85] = (seg[:128, None] <= seg[None, :85]).astype(np.float32)
    consts = np.concatenate([PI, I128, ones, bcm, bcm], axis=1)  # [128, 576]
    return tabs.astype(BF16), consts.astype(BF16)


def _build(mask_mode, use_qkv_bias):
    import concourse.bass as bass
    import concourse.bacc as bacc
    import concourse.tile as tile
    from concourse import mybir

    f32, bf16 = mybir.dt.float32, mybir.dt.bfloat16
    f8 = mybir.dt.float8e4
    AF = mybir.ActivationFunctionType
    DR = mybir.MatmulPerfMode.DoubleRow
    nc = bacc.Bacc("TRN2", target_bir_lowering=False, debug=False)

    xt_d = nc.dram_tensor("xt", [DIM, T], bf16, kind="ExternalInput")
    if FP8_QK:
        xt8_d = nc.dram_tensor("xt8", [DIM, T], f8, kind="ExternalInput")
        wqk8_d = nc.dram_tensor("wqk8", [DIM, 2048], f8, kind="ExternalInput")
    else:
        wqk_d = nc.dram_tensor("wqk", [DIM, 2048], bf16, kind="ExternalInput")
    wv_d = nc.dram_tensor("wv", [DIM, 1024], bf16, kind="ExternalInput")
    wp_d = nc.dram_tensor("wp", [DIM, 1024], bf16, kind="ExternalInput")
    tabs_d = nc.dram_tensor("tabs", [128, 8 * NP], bf16, kind="ExternalInput")
    consts_d = nc.dram_tensor("consts", [128, 576], bf16, kind="ExternalInput")
    general = mask_mode == "general"
    if general:
        maskm_d = nc.dram_tensor("maskm", [128, 3 * NP], bf16, kind="ExternalInput")
    if use_qkv_bias:
        qb_d = nc.dram_tensor("qb", [128, 16 * NP], bf16, kind="ExternalInput")
        vb_d = nc.dram_tensor("vb", [1, 1024], bf16, kind="ExternalInput")
    out_d = nc.dram_tensor("out", [BPC * NTOK, DIM], bf16, kind="ExternalOutput")

    bc = mask_mode == "bc"
    # ex free-dim layout: bc packs si1+si2 as one 512-col block
    EXW = NP + 2 * 256 if bc else 3 * NP
    EXOFF = [0, NP, NP + 256] if bc else [0, NP, 2 * NP]

    with tile.TileContext(nc) as tc, \
         nc.allow_low_precision(reason="bf16/fp8 qk; rel gate 2e-2"):
        with tc.tile_pool(name="res", bufs=1) as res, \
             tc.tile_pool(name="vp", bufs=9) as vpool, \
             tc.tile_pool(name="qkp", bufs=2) as qkpool, \
             tc.tile_pool(name="ro", bufs=2) as ropool, \
             tc.tile_pool(name="ex", bufs=2) as expool, \
             tc.tile_pool(name="asb", bufs=2) as asbp, \
             tc.tile_pool(name="st", bufs=1) as stpool, \
             tc.tile_pool(name="at", bufs=2) as atpool, \
             tc.tile_pool(name="prb", bufs=1) as prbp, \
             tc.tile_pool(name="ys", bufs=2) as yspool, \
             tc.tile_pool(name="dr", bufs=1, space="DRAM") as drp, \
             tc.tile_pool(name="sa", bufs=4, space="PSUM") as sap, \
             tc.tile_pool(name="av", bufs=2, space="PSUM") as avp, \
             tc.tile_pool(name="hf", bufs=2, space="PSUM") as hfp:

            # ---- resident loads (order matters: v(0..2) cover the rest) ----
            xt = res.tile([128, 8, T], bf16)
            if FP8_QK:
                xt8 = res.tile([128, 8, T], f8)
                wqk = res.tile([128, 8, 2048], f8)
            else:
                wqk = res.tile([128, 8, 2048], bf16)
            wv = res.tile([128, 8, 1024], bf16)
            wp = res.tile([128, 8, 1024], bf16)
            for c in range(8):
                nc.sync.dma_start(wv[:, c, :], wv_d[c * 128:(c + 1) * 128, :])
                nc.sync.dma_start(xt[:, c, 0:NP], xt_d[c * 128:(c + 1) * 128, 0:NP])
            for b in range(1, 3):
                for c in range(8):
                    nc.sync.dma_start(xt[:, c, b * NP:(b + 1) * NP],
                                      xt_d[c * 128:(c + 1) * 128, b * NP:(b + 1) * NP])
            tabs = res.tile([128, 8, NP], bf16)
            nc.sync.dma_start(tabs[:], tabs_d[:])
            consts = res.tile([128, 576], bf16)
            nc.sync.dma_start(consts[:], consts_d[:])
            for c in range(8):
                if FP8_QK:
                    nc.sync.dma_start(wqk[:, c, :], wqk8_d[c * 128:(c + 1) * 128, :])
                    nc.sync.dma_start(xt8[:, c, :], xt8_d[c * 128:(c + 1) * 128, :])
                else:
                    nc.sync.dma_start(wqk[:, c, :], wqk_d[c * 128:(c + 1) * 128, :])
            for c in range(8):
                nc.sync.dma_start(wp[:, c, :], wp_d[c * 128:(c + 1) * 128, :])
            for c in range(8):
                nc.sync.dma_start(xt[:, c, 3 * NP:T],
                                  xt_d[c * 128:(c + 1) * 128, 3 * NP:T])
            if general:
                maskm = res.tile([128, 3, NP], bf16)
                nc.sync.dma_start(maskm[:], maskm_d[:])
            if use_qkv_bias:
                qb = res.tile([128, 16, NP], bf16)
                nc.sync.dma_start(qb[:], qb_d[:])
                vb = res.tile([1, 1024], bf16)
                nc.sync.dma_start(vb[:], vb_d[:])

            PI = consts[:, 0:128]
            I128 = consts[:, 128:256]
            bcm = consts[:, 384:480]

            def sa_tile(name):
                return sap.tile([128, 512], f32, tag="sa", name=name)

            vt = {}     # b -> [v_s tiles per slice]
            att = {}    # b -> att tile
            prb = {}    # b -> broadcast reciprocal sums [64, 16, NP]
            asba = {}   # b -> asb_all [128, 16, NP]

            def emit_v(b):
                boff = b * NP
                vts = []
                for s, (t0, tsz) in enumerate(MSL):
                    v_s = vpool.tile([128, 16, 65], bf16, name="v_s")
                    for half in range(2):
                        pv = hfp.tile([128, 512], f32, tag="hf", name="pv")
                        for c in range(8):
                            nc.tensor.matmul(
                                pv[0:tsz, :],
                                lhsT=xt[:, c, boff + t0: boff + t0 + tsz],
                                rhs=wv[:, c, half * 512:(half + 1) * 512],
                                start=(c == 0), stop=(c == 7 and not use_qkv_bias))
                        if use_qkv_bias:
                            nc.tensor.matmul(
                                pv[0:tsz, :],
                                lhsT=consts[0:1, 256:256 + tsz],  # row of ones
                                rhs=vb[:, half * 512:(half + 1) * 512],
                                start=False, stop=True)
                        nc.vector.tensor_copy(
                            v_s[0:tsz, half * 8:(half + 1) * 8, 0:64], pv[0:tsz, :])
                    nc.vector.memset(v_s[:, :, 64:65], 1.0)
                    vts.append(v_s)
                vt[b] = vts

            def emit_rot(f, umul, tmul, qk):
                # deferred rope for feature tile f: pair-swap matmul + add
                prot = sa_tile("prot")
                nc.tensor.matmul(prot[:, 0:NP], lhsT=PI, rhs=umul[:],
                                 start=True, stop=not use_qkv_bias)
                if use_qkv_bias:
                    nc.tensor.matmul(prot[:, 0:NP], lhsT=I128,
                                     rhs=qb[:, f, :], start=False, stop=True)
                nc.vector.tensor_tensor(qk[:, f, :], prot[:, 0:NP], tmul[:],
                                        mybir.AluOpType.add)

            def emit_qk(b):
                boff = b * NP
                qk = qkpool.tile([128, 16, NP], bf16, name="qk")
                pend = None
                # q/k tiles interleaved so head-pair p's q AND k tiles
                # complete early, in p order
                for f in (0, 8, 1, 9, 2, 10, 3, 11, 4, 12, 5, 13, 6, 14, 7, 15):
                    pq = sa_tile("pq")
                    for c in range(8):
                        nc.tensor.matmul(
                            pq[:, 0:NP],
                            lhsT=wqk[:, c, f * 128:(f + 1) * 128],
                            rhs=xt[:, c, boff: boff + NP],
                            start=(c == 0), stop=(c == 7))
                    if pend is not None:
                        emit_rot(*pend, qk)
                    tb = 0 if f < 8 else 4
                    cosT = tabs[:, tb, :]
                    sinT = tabs[:, tb + 2, :]
                    qsb = ropool.tile([128, NP], bf16, tag="qs", name="qsb")
                    nc.scalar.copy(qsb[:], pq[:, 0:NP])
                    tmul = ropool.tile([128, NP], bf16, tag="tm", name="tmul")
                    nc.vector.tensor_tensor(tmul[:], qsb[:], cosT,
                                            mybir.AluOpType.mult)
                    umul = ropool.tile([128, NP], bf16, tag="um", name="umul")
                    nc.vector.tensor_tensor(umul[:], qsb[:], sinT,
                                            mybir.AluOpType.mult)
                    pend = (f, umul, tmul)
                emit_rot(*pend, qk)
                return qk

            def emit_scores(qk, p):
                # hh pairs emitted adjacently: the K=64 row-tiled matmuls run
                # concurrently on the PE
                def mm(out, msel, nsel, hh):
                    r0 = hh * 64
                    nc.tensor.matmul(
                        out,
                        lhsT=qk[r0:r0 + 64, 8 + p, msel[0]:msel[1]],
                        rhs=qk[r0:r0 + 64, p, nsel[0]:nsel[1]],
                        start=True, stop=True, tile_position=(r0, 0))
                s0 = [sa_tile("s0") for _ in range(2)]
                for hh in range(2):
                    mm(s0[hh][0:128, 0:NP], (0, 128), (0, NP), hh)
                if bc:
                    s12 = [sa_tile("s12") for _ in range(2)]
                    for hh in range(2):
                        mm(s12[hh][0:128, 0:NW], (128, 256), (85, 341), hh)
                    for hh in range(2):
                        mm(s12[hh][0:85, NW:2 * NW], (256, 341), (85, 341), hh)
                    return [(s0[hh], s12[hh]) for hh in range(2)]
                else:
                    s = [[sa_tile(f"s{si}") for _ in range(2)] for si in (1, 2)]
                    for k, (m0, msz) in enumerate(MSL[1:]):
                        for hh in range(2):
                            mm(s[k][hh][0:msz, 0:NP], (m0, m0 + msz), (0, NP), hh)
                    return [(s0[hh], s[0][hh], s[1][hh]) for hh in range(2)]

            def emit_exp(slots, ex, hh):
                if bc:
                    s0, s12 = slots
                    nc.scalar.activation(ex[0:128, 0:NP], s0[0:128, 0:NP], AF.Exp)
                    nc.vector.tensor_tensor(ex[0:128, 0:85], ex[0:128, 0:85],
                                            bcm[0:128, 0:85],
                                            mybir.AluOpType.mult)
                    nc.scalar.activation(ex[0:128, NP:NP + 512],
                                         s12[0:128, 0:512], AF.Exp)
                else:
                    for si, (m0, msz) in enumerate(MSL):
                        o = EXOFF[si]
                        if general:
                            exr = ropool.tile([128, NP], bf16, tag=f"exr{hh}",
                                              name="exr")
                            nc.scalar.activation(exr[0:msz, :],
                                                 slots[si][0:msz, 0:NP], AF.Exp)
                            nc.vector.tensor_tensor(
                                ex[0:msz, o:o + NP],
                                exr[0:msz, :], maskm[0:msz, si, :],
                                mybir.AluOpType.mult)
                        else:
                            nc.scalar.activation(
                                ex[0:msz, o:o + NP],
                                slots[si][0:msz, 0:NP], AF.Exp)

            def emit_av(b, p, exs):
                for hh in range(2):
                    h = 2 * p + hh
                    ex = exs[hh]
                    pav = avp.tile([128, 512], f32, tag="av", name="pav")
                    if bc:
                        nc.tensor.matmul(
                            pav[0:65, 0:NP], lhsT=vt[b][0][0:128, h, :],
                            rhs=ex[0:128, 0:NP], start=True, stop=False)
                        nc.tensor.matmul(
                            pav[0:65, 85:341], lhsT=vt[b][1][0:128, h, :],
                            rhs=ex[0:128, NP:NP + NW], start=False, stop=False)
                        nc.tensor.matmul(
                            pav[0:65, 85:341], lhsT=vt[b][2][0:85, h, :],
                            rhs=ex[0:85, NP + NW:NP + 2 * NW],
                            start=False, stop=True)
                    else:
                        for si, (m0, msz) in enumerate(MSL):
                            nc.tensor.matmul(
                                pav[0:65, 0:NP], lhsT=vt[b][si][0:msz, h, :],
                                rhs=ex[0:msz, EXOFF[si]:EXOFF[si] + NP],
                                start=(si == 0), stop=(si == 2))
                    nc.scalar.copy(asba[b][0:65, h, :], pav[0:65, 0:NP])

            def emit_sc(b, qk, finb):
                asba[b] = asbp.tile([128, 16, NP], bf16, name="asb_all")
                if finb is not None:
                    att[finb] = atpool.tile([128, 8, NP], bf16, name="att")
                pend = None
                for p in range(8):
                    exs = [expool.tile([128, EXW], bf16, tag=f"e{hh}", name="ex")
                           for hh in range(2)]
                    slots = emit_scores(qk, p)
                    for hh in range(2):
                        emit_exp(slots[hh], exs[hh], hh)
                    if finb is not None:
                        for hh in range(2):
                            nc.vector.tensor_tensor(
                                att[finb][hh * 64:(hh + 1) * 64, p, :],
                                asba[finb][0:64, 2 * p + hh, :],
                                prb[finb][0:64, 2 * p + hh, :],
                                mybir.AluOpType.mult)
                    if pend is not None:
                        emit_av(b, *pend)
                    pend = (p, exs)
                emit_av(b, *pend)

            def emit_rec(b):
                staged = stpool.tile([16, NP], bf16, tag="staged", name="staged")
                nc.sync.dma_start(staged[:], asba[b][64:65, :, :])
                rec = stpool.tile([16, NP], bf16, tag="rec", name="rec")
                nc.vector.reciprocal(rec[:], staged[:])
                # bounce through DRAM, then broadcast each head's row across
                # 64 partitions with step-0 DMAs
                rd = drp.tile([16, NP], bf16, tag="rd", name="rd")
                nc.sync.dma_start(rd[:], rec[:])
                prb[b] = prbp.tile([64, 16, NP], bf16, name="prb")
                for h in range(16):
                    nc.sync.dma_start(prb[b][0:64, h, :],
                                      rd[h:h + 1, :].to_broadcast([64, NP]))

            def emit_fin(b):
                # standalone normalization (last batch)
                for p in range(8):
                    for hh in range(2):
                        nc.vector.tensor_tensor(
                            att[b][hh * 64:(hh + 1) * 64, p, :],
                            asba[b][0:64, 2 * p + hh, :],
                            prb[b][0:64, 2 * p + hh, :],
                            mybir.AluOpType.mult)

            def emit_proj(b):
                for s, (t0, tsz) in enumerate(MSL):
                    rsz = min(tsz, NTOK - t0)
                    ysb = yspool.tile([128, 1024], bf16, name="ysb")
                    for half in range(2):
                        py = hfp.tile([128, 512], f32, tag="hf", name="py")
                        for c in range(8):
                            nc.tensor.matmul(
                                py[0:tsz, :],
                                lhsT=att[b][:, c, t0:t0 + tsz],
                                rhs=wp[:, c, half * 512:(half + 1) * 512],
                                start=(c == 0), stop=(c == 7))
                        nc.scalar.copy(ysb[0:tsz, half * 512:(half + 1) * 512],
                                       py[0:tsz, :])
                    nc.sync.dma_start(
                        out_d[b * NTOK + t0: b * NTOK + t0 + rsz, :],
                        ysb[0:rsz, :])

            # ---- schedule ----
            emit_v(0); emit_v(1); emit_v(2)
            qk0 = emit_qk(0)
            emit_sc(0, qk0, None); emit_rec(0)
            emit_v(3)
            qk1 = emit_qk(1)
            emit_sc(1, qk1, 0); emit_rec(1)
            emit_proj(0)
            qk2 = emit_qk(2)
            emit_sc(2, qk2, 1); emit_rec(2)
            emit_proj(1)
            qk3 = emit_qk(3)
            emit_sc(3, qk3, 2); emit_rec(3)
            emit_proj(2)
            att[3] = atpool.tile([128, 8, NP], bf16, name="att3")
            emit_fin(3)
            emit_proj(3)
    nc.finalize()
    return nc


def _get_nc(mask_mode, use_qkv_bias):
    key = (mask_mode, use_qkv_bias)
    if key not in _cache:
        _cache[key] = _build(mask_mode, use_qkv_bias)
    return _cache[key]


def _bc_mask():
    seg = np.concatenate([np.full(s * s, i, dtype=np.int64) for i, s in enumerate(SCALES)])
    allow = seg[:, None] >= seg[None, :]
    return np.where(allow, 0.0, -1e9).astype(np.float32)[None, None]


def _prep_core_inputs(x, mask, qkv_w, qkv_b, proj_w, proj_b):
    tabs, consts = _host_tables()
    mf = mask.astype(np.float32)
    if not np.any(mf != 0):
        mask_mode = "none"
    elif np.array_equal(mf, _bc_mask()):
        mask_mode = "bc"
    else:
        mask_mode = "general"
    use_qb = bool(np.any(qkv_b != 0))

    wqkT = qkv_w.astype(np.float32).T                    # [1024, 3072]
    wv = np.ascontiguousarray(wqkT[:, 2048:].astype(BF16))
    wpT = np.ascontiguousarray(proj_w.astype(np.float32).T.astype(BF16))

    common = {"wv": wv, "wp": wpT, "tabs": np.ascontiguousarray(tabs),
              "consts": np.ascontiguousarray(consts)}
    if FP8_QK:
        common["wqk8"] = np.ascontiguousarray(
            (wqkT[:, :2048] * W8SCALE).astype(F8))
    else:
        common["wqk"] = np.ascontiguousarray(wqkT[:, :2048].astype(BF16))
    if mask_mode == "general":
        mT = mask[0, 0].astype(np.float32).T            # [keys, queries]
        mm = np.zeros((384, NP), np.float32)
        mm[:NTOK, :NTOK] = np.exp(mT)                   # multiplicative mask
        maskm = np.zeros((128, 3 * NP), np.float32)
        for s in range(3):
            maskm[:, s * NP:(s + 1) * NP] = mm[s * 128:(s + 1) * 128, :]
        common["maskm"] = maskm.astype(BF16)
    if use_qb:
        cos, sin = _rope_tables()
        sin2 = sin.copy(); sin2[:, 0::2] = -sin[:, 0::2]
        scale = 1.0 / np.sqrt(HD)
        qb_full = np.zeros((128, 16 * NP), np.float32)
        bq = qkv_b[:2048].astype(np.float32)
        for f in range(16):
            is_q = f < 8
            sc = scale if is_q else 1.0
            for hh in range(2):
                hvec = bq[f * 128 + hh * 64: f * 128 + (hh + 1) * 64]  # [64]
                hswap = hvec.reshape(-1, 2)[:, ::-1].reshape(-1)
                rb = cos * hvec[None, :] + sin2 * hswap[None, :]       # [341,64]
                qb_full[hh * 64:(hh + 1) * 64, f * NP: f * NP + NTOK] = sc * rb.T
        common["qb"] = qb_full.astype(BF16)
        common["vb"] = qkv_b[2048:].astype(np.float32).astype(BF16)[None, :]

    in_maps = []
    xf = x.astype(np.float32)
    for core in range(NCORES):
        xc = xf[core * BPC:(core + 1) * BPC]            # [4, 341, 1024]
        xp = np.zeros((BPC, NP, DIM), np.float32)
        xp[:, :NTOK, :] = xc
        xT = np.ascontiguousarray(xp.reshape(BPC * NP, DIM).T)  # [1024, 1376]
        m = dict(common)
        m["xt"] = xT.astype(BF16)
        if FP8_QK:
            m["xt8"] = xT.astype(F8)
        in_maps.append(m)
    return in_maps, mask_mode, use_qb


def kernel(x, mask, qkv_w, qkv_b, proj_w, proj_b, _trace=False):
    from concourse.bass_utils import run_bass_kernel_spmd
    x, mask, qkv_w, qkv_b, proj_w, proj_b = (
        np.asarray(t) for t in (x, mask, qkv_w, qkv_b, proj_w, proj_b))
    in_maps, mask_mode, use_qb = _prep_core_inputs(
        x, mask, qkv_w, qkv_b, proj_w, proj_b)
    nc = _get_nc(mask_mode, use_qb)
    res = run_bass_kernel_spmd(nc, in_maps, core_ids=list(range(NCORES)),
                               trace=_trace)
    out = np.empty((B, NTOK, DIM), np.float32)
    for core in range(NCORES):
        y = res.results[core]["out"].astype(np.float32).reshape(BPC, NTOK, DIM)
        out[core * BPC:(core + 1) * BPC] = y
    pb = proj_b.astype(np.float32)
    if np.any(pb != 0):
        out += pb[None, None, :]
    kernel._last_exec_time_ns = res.exec_time_ns
    kernel._last_results = res
    return out


# revision 39
# speedup vs baseline: 1.1586x; 1.0268x over previous
"""Trainium2 Bass kernel for a 16-head attention block with 2D axial RoPE.

Strategy: pure data-parallel over batch (32 batches -> 4 per NeuronCore),
bf16 compute with an fp8 (DoubleRow) QKV q/k projection, feature-major
("transposed") layouts throughout:
  - q/k stay feature-major from the QKV projection (fp8 weights x256 with
    the 1/256 folded into the rope tables; errors attenuate through
    softmax); v is produced token-major directly in bf16.
  - RoPE via two elementwise muls + a pair-swap permutation matmul;
    feature tiles processed in pairs sharing [128,2,512] PSUM tiles so
    every elementwise op covers two tiles in one instruction.
  - scoresT[m,n] per head (keys on partitions): row-packed K=64 matmul
    pairs via tile_position writing the two halves of one PSUM pair
    tile; si1+si2 share one bank; exp runs as two activations per head
    pair; block-causal mask applied as one small 0/1-mask multiply over
    query columns 0:85; softmax without max subtraction; per-query sums
    via an appended ones-column on v; normalization via DMA-broadcast
    reciprocal sums + one bf16 multiply per head.
  - proj maps back to token-major; bf16 output DMA.
"""
import sys, os
sys.path.insert(0, "/opt/trn_rl_repo")
import numpy as np
import ml_dtypes

B, NTOK, DIM, H, HD = 32, 341, 1024, 16, 64
NCORES, BPC = 8, 4          # cores, batches per core
NP = 344                    # padded tokens per batch (bf16 pair aligned)
T = BPC * NP                # 1376 tokens per core
SCALES = [1, 2, 4, 8, 16]
PT_SEQ_LEN, THETA = 16, 10000.0
ROPE_DIM = HD // 2
MSL = [(0, 128), (128, 128), (256, 85)]   # m/token slices per batch
NW = 341 - 85               # si>0 query window size (= 256, one half-bank)
FP8_QK = False              # fp8 DoubleRow q/k projection (fails 2e-2 gate)
W8SCALE = 256.0
BF16 = ml_dtypes.bfloat16
F8 = ml_dtypes.float8_e4m3fn

_cache = {}


def _rope_tables():
    inv = 1.0 / (THETA ** (np.arange(0, ROPE_DIM, 2, dtype=np.float64) / ROPE_DIM))
    cos_list, sin_list = [], []
    for s in SCALES:
        t = np.arange(s, dtype=np.float64) / s * PT_SEQ_LEN
        f = np.outer(t, inv)
        f = np.repeat(f, 2, axis=-1)
        fy = np.broadcast_to(f[:, None, :], (s, s, ROPE_DIM))
        fx = np.broadcast_to(f[None, :, :], (s, s, ROPE_DIM))
        ff = np.concatenate([fy, fx], axis=-1).reshape(s * s, HD)
        cos_list.append(np.cos(ff))
        sin_list.append(np.sin(ff))
    cos = np.concatenate(cos_list, axis=0).astype(np.float32)  # [341, 64]
    sin = np.concatenate(sin_list, axis=0).astype(np.float32)
    return cos, sin


def _host_tables():
    cos, sin = _rope_tables()               # [341, 64]
    # sin2: sign pattern for rotate_half: q'[2i] = q[2i]c - q[2i+1]s ...
    sin2 = sin.copy()
    sin2[:, 0::2] = -sin[:, 0::2]
    # sinP[e] = sin2[e^1] (so that (PI @ (q*sinP))[d] = q[d^1]*sin2[d])
    sinP = np.empty_like(sin2)
    sinP[:, 0::2] = sin2[:, 1::2]
    sinP[:, 1::2] = sin2[:, 0::2]
    cosT = np.zeros((HD, NP), np.float32)
    sinPT = np.zeros((HD, NP), np.float32)
    cosT[:, :NTOK] = cos.T
    sinPT[:, :NTOK] = sinP.T
    cos128 = np.vstack([cosT, cosT])        # [128, NP] two heads per tile
    sinP128 = np.vstack([sinPT, sinPT])
    qs = (1.0 / np.sqrt(HD)) / (W8SCALE if FP8_QK else 1.0)
    ks = 1.0 / (W8SCALE if FP8_QK else 1.0)
    # tabs: [cosq, cosq, sinq, sinq, cosk, cosk, sink, sink] (f-pair duplicated)
    blocks = [cos128 * qs, cos128 * qs, sinP128 * qs, sinP128 * qs,
              cos128 * ks, cos128 * ks, sinP128 * ks, sinP128 * ks]
    tabs = np.concatenate(blocks, axis=1)   # [128, 8*NP]
    # consts: PI | I128 | ones | bcm2 (duplicated per head-pair)
    PI = np.zeros((128, 128), np.float32)
    for d in range(128):
        PI[d ^ 1, d] = 1.0
    I128 = np.eye(128, dtype=np.float32)
    ones = np.ones((128, 128), dtype=np.float32)
    seg = np.concatenate([np.full(s * s, i, dtype=np.int64) for i, s in enumerate(SCALES)])
    bcm = np.ones((128, 96), dtype=np.float32)
    bcm[:, 0:85] = (seg[:128, None] <= seg[None, :85]).astype(np.float32)
    consts = np.concatenate([PI, I128, ones, bcm, bcm], axis=1)  # [128, 576]
    return tabs.astype(BF16), consts.astype(BF16)


def _build(mask_mode, use_qkv_bias):
    import concourse.bass as bass
    import concourse.bacc as bacc
    import concourse.tile as tile
    from concourse import mybir

    f32, bf16 = mybir.dt.float32, mybir.dt.bfloat16
    f8 = mybir.dt.float8e4
    AF = mybir.ActivationFunctionType
    DR = mybir.MatmulPerfMode.DoubleRow
    nc = bacc.Bacc("TRN2", target_bir_lowering=False, debug=False)

    xt_d = nc.dram_tensor("xt", [DIM, T], bf16, kind="ExternalInput")
    if FP8_QK:
        xt8_d = nc.dram_tensor("xt8", [DIM, T], f8, kind="ExternalInput")
        wqk8_d = nc.dram_tensor("wqk8", [DIM, 2048], f8, kind="ExternalInput")
    else:
        wqk_d = nc.dram_tensor("wqk", [DIM, 2048], bf16, kind="ExternalInput")
    wv_d = nc.dram_tensor("wv", [DIM, 1024], bf16, kind="ExternalInput")
    wp_d = nc.dram_tensor("wp", [DIM, 1024], bf16, kind="ExternalInput")
    tabs_d = nc.dram_tensor("tabs", [128, 8 * NP], bf16, kind="ExternalInput")
    consts_d = nc.dram_tensor("consts", [128, 576], bf16, kind="ExternalInput")
    general = mask_mode == "general"
    if general:
        maskm_d = nc.dram_tensor("maskm", [128, 3 * NP], bf16, kind="ExternalInput")
    if use_qkv_bias:
        qb_d = nc.dram_tensor("qb", [128, 16 * NP], bf16, kind="ExternalInput")
        vb_d = nc.dram_tensor("vb", [1, 1024], bf16, kind="ExternalInput")
    out_d = nc.dram_tensor("out", [BPC * NTOK, DIM], bf16, kind="ExternalOutput")

    bc = mask_mode == "bc"
    # ex free-dim layout: bc packs si1+si2 as one 512-col block
    EXW = NP + 2 * 256 if bc else 3 * NP
    EXOFF = [0, NP, NP + 256] if bc else [0, NP, 2 * NP]

    with tile.TileContext(nc) as tc, \
         nc.allow_low_precision(reason="bf16/fp8 qk; rel gate 2e-2"):
        with tc.tile_pool(name="res", bufs=1) as res, \
             tc.tile_pool(name="vp", bufs=9) as vpool, \
             tc.tile_pool(name="qkp", bufs=2) as qkpool, \
             tc.tile_pool(name="ro", bufs=2) as ropool, \
             tc.tile_pool(name="ex", bufs=2) as expool, \
             tc.tile_pool(name="asb", bufs=2) as asbp, \
             tc.tile_pool(name="st", bufs=1) as stpool, \
             tc.tile_pool(name="at", bufs=2) as atpool, \
             tc.tile_pool(name="prb", bufs=1) as prbp, \
             tc.tile_pool(name="ys", bufs=2) as yspool, \
             tc.tile_pool(name="dr", bufs=1, space="DRAM") as drp, \
             tc.tile_pool(name="sa", bufs=4, space="PSUM") as sap, \
             tc.tile_pool(name="av", bufs=2, space="PSUM") as avp, \
             tc.tile_pool(name="hf", bufs=2, space="PSUM") as hfp:

            # ---- resident loads (order matters: v(0..2) cover the rest) ----
            xt = res.tile([128, 8, T], bf16)
            if FP8_QK:
                xt8 = res.tile([128, 8, T], f8)
                wqk = res.tile([128, 8, 2048], f8)
            else:
                wqk = res.tile([128, 8, 2048], bf16)
            wv = res.tile([128, 8, 1024], bf16)
            wp = res.tile([128, 8, 1024], bf16)
            for c in range(8):
                nc.sync.dma_start(wv[:, c, :], wv_d[c * 128:(c + 1) * 128, :])
                nc.sync.dma_start(xt[:, c, 0:NP], xt_d[c * 128:(c + 1) * 128, 0:NP])
            for b in range(1, 3):
                for c in range(8):
                    nc.sync.dma_start(xt[:, c, b * NP:(b + 1) * NP],
                                      xt_d[c * 128:(c + 1) * 128, b * NP:(b + 1) * NP])
            tabs = res.tile([128, 8, NP], bf16)
            nc.sync.dma_start(tabs[:], tabs_d[:])
            consts = res.tile([128, 576], bf16)
            nc.sync.dma_start(consts[:], consts_d[:])
            for c in range(8):
                if FP8_QK:
                    nc.sync.dma_start(wqk[:, c, :], wqk8_d[c * 128:(c + 1) * 128, :])
                    nc.sync.dma_start(xt8[:, c, :], xt8_d[c * 128:(c + 1) * 128, :])
                else:
                    nc.sync.dma_start(wqk[:, c, :], wqk_d[c * 128:(c + 1) * 128, :])
            for c in range(8):
                nc.sync.dma_start(wp[:, c, :], wp_d[c * 128:(c + 1) * 128, :])
            for c in range(8):
                nc.sync.dma_start(xt[:, c, 3 * NP:T],
                                  xt_d[c * 128:(c + 1) * 128, 3 * NP:T])
            if general:
                maskm = res.tile([128, 3, NP], bf16)
                nc.sync.dma_start(maskm[:], maskm_d[:])
            if use_qkv_bias:
                qb = res.tile([128, 16, NP], bf16)
                nc.sync.dma_start(qb[:], qb_d[:])
                vb = res.tile([1, 1024], bf16)
                nc.sync.dma_start(vb[:], vb_d[:])

            PI = consts[:, 0:128]
            I128 = consts[:, 128:256]
            bcm = consts[:, 384:480]

            def sa_tile(name):
                return sap.tile([128, 512], f32, tag="sa", name=name)

            vt = {}     # b -> [v_s tiles per slice]
            att = {}    # b -> att tile
            prb = {}    # b -> broadcast reciprocal sums [64, 16, NP]
            asba = {}   # b -> asb_all [128, 16, NP]

            def emit_v(b):
                boff = b * NP
                vts = []
                for s, (t0, tsz) in enumerate(MSL):
                    v_s = vpool.tile([128, 16, 65], bf16, name="v_s")
                    for half in range(2):
                        pv = hfp.tile([128, 512], f32, tag="hf", name="pv")
                        for c in range(8):
                            nc.tensor.matmul(
                                pv[0:tsz, :],
                                lhsT=xt[:, c, boff + t0: boff + t0 + tsz],
                                rhs=wv[:, c, half * 512:(half + 1) * 512],
                                start=(c == 0), stop=(c == 7 and not use_qkv_bias))
                        if use_qkv_bias:
                            nc.tensor.matmul(
                                pv[0:tsz, :],
                                lhsT=consts[0:1, 256:256 + tsz],  # row of ones
                                rhs=vb[:, half * 512:(half + 1) * 512],
                                start=False, stop=True)
                        nc.vector.tensor_copy(
                            v_s[0:tsz, half * 8:(half + 1) * 8, 0:64], pv[0:tsz, :])
                    nc.vector.memset(v_s[:, :, 64:65], 1.0)
                    vts.append(v_s)
                vt[b] = vts

            def emit_rot(f, umul, tmul, qk):
                # deferred rope for feature tile f: pair-swap matmul + add
                prot = sa_tile("prot")
                nc.tensor.matmul(prot[:, 0:NP], lhsT=PI, rhs=umul[:],
                                 start=True, stop=not use_qkv_bias)
                if use_qkv_bias:
                    nc.tensor.matmul(prot[:, 0:NP], lhsT=I128,
                                     rhs=qb[:, f, :], start=False, stop=True)
                nc.vector.tensor_tensor(qk[:, f, :], prot[:, 0:NP], tmul[:],
                                        mybir.AluOpType.add)

            def emit_qk(b):
                boff = b * NP
                qk = qkpool.tile([128, 16, NP], bf16, name="qk")
                pend = None
                # q/k tiles interleaved so head-pair p's q AND k tiles
                # complete early, in p order
                for f in (0, 8, 1, 9, 2, 10, 3, 11, 4, 12, 5, 13, 6, 14, 7, 15):
                    pq = sa_tile("pq")
                    for c in range(8):
                        nc.tensor.matmul(
                            pq[:, 0:NP],
                            lhsT=wqk[:, c, f * 128:(f + 1) * 128],
                            rhs=xt[:, c, boff: boff + NP],
                            start=(c == 0), stop=(c == 7))
                    if pend is not None:
                        emit_rot(*pend, qk)
                    tb = 0 if f < 8 else 4
                    cosT = tabs[:, tb, :]
                    sinT = tabs[:, tb + 2, :]
                    qsb = ropool.tile([128, NP], bf16, tag="qs", name="qsb")
                    nc.scalar.copy(qsb[:], pq[:, 0:NP])
                    tmul = ropool.tile([128, NP], bf16, tag="tm", name="tmul")
                    nc.vector.tensor_tensor(tmul[:], qsb[:], cosT,
                                            mybir.AluOpType.mult)
                    umul = ropool.tile([128, NP], bf16, tag="um", name="umul")
                    nc.vector.tensor_tensor(umul[:], qsb[:], sinT,
                                            mybir.AluOpType.mult)
                    pend = (f, umul, tmul)
                emit_rot(*pend, qk)
                return qk

            def emit_scores(qk, p):
                # hh pairs emitted adjacently: the K=64 row-tiled matmuls run
                # concurrently on the PE
                def mm(out, msel, nsel, hh):
                    r0 = hh * 64
                    nc.tensor.matmul(
                        out,
                        lhsT=qk[r0:r0 + 64, 8 + p, msel[0]:msel[1]],
                        rhs=qk[r0:r0 + 64, p, nsel[0]:nsel[1]],
                        start=True, stop=True, tile_position=(r0, 0))
                s0 = [sa_tile("s0") for _ in range(2)]
                for hh in range(2):
                    mm(s0[hh][0:128, 0:NP], (0, 128), (0, NP), hh)
                if bc:
                    s12 = [sa_tile("s12") for _ in range(2)]
                    for hh in range(2):
                        mm(s12[hh][0:128, 0:NW], (128, 256), (85, 341), hh)
                    for hh in range(2):
                        mm(s12[hh][0:85, NW:2 * NW], (256, 341), (85, 341), hh)
                    return [(s0[hh], s12[hh]) for hh in range(2)]
                else:
                    s = [[sa_tile(f"s{si}") for _ in range(2)] for si in (1, 2)]
                    for k, (m0, msz) in enumerate(MSL[1:]):
                        for hh in range(2):
                            mm(s[k][hh][0:msz, 0:NP], (m0, m0 + msz), (0, NP), hh)
                    return [(s0[hh], s[0][hh], s[1][hh]) for hh in range(2)]

            def emit_exp(slots, ex, hh):
                if bc:
                    s0, s12 = slots
                    nc.scalar.activation(ex[0:128, 0:NP], s0[0:128, 0:NP], AF.Exp)
                    nc.vector.tensor_tensor(ex[0:128, 0:85], ex[0:128, 0:85],
                                            bcm[0:128, 0:85],
                                            mybir.AluOpType.mult)
                    nc.scalar.activation(ex[0:128, NP:NP + 512],
                                         s12[0:128, 0:512], AF.Exp)
                else:
                    for si, (m0, msz) in enumerate(MSL):
                        o = EXOFF[si]
                        if general:
                            exr = ropool.tile([128, NP], bf16, tag=f"exr{hh}",
                                              name="exr")
                            nc.scalar.activation(exr[0:msz, :],
                                                 slots[si][0:msz, 0:NP], AF.Exp)
                            nc.vector.tensor_tensor(
                                ex[0:msz, o:o + NP],
                                exr[0:msz, :], maskm[0:msz, si, :],
                                mybir.AluOpType.mult)
                        else:
                            nc.scalar.activation(
                                ex[0:msz, o:o + NP],
                                slots[si][0:msz, 0:NP], AF.Exp)

            def emit_av(b, p, exs):
                for hh in range(2):
                    h = 2 * p + hh
                    ex = exs[hh]
                    pav = avp.tile([128, 512], f32, tag="av", name="pav")
                    if bc:
                        nc.tensor.matmul(
                            pav[0:65, 0:NP], lhsT=vt[b][0][0:128, h, :],
                            rhs=ex[0:128, 0:NP], start=True, stop=False)
                        nc.tensor.matmul(
                            pav[0:65, 85:341], lhsT=vt[b][1][0:128, h, :],
                            rhs=ex[0:128, NP:NP + NW], start=False, stop=False)
                        nc.tensor.matmul(
                            pav[0:65, 85:341], lhsT=vt[b][2][0:85, h, :],
                            rhs=ex[0:85, NP + NW:NP + 2 * NW],
                            start=False, stop=True)
                    else:
                        for si, (m0, msz) in enumerate(MSL):
                            nc.tensor.matmul(
                                pav[0:65, 0:NP], lhsT=vt[b][si][0:msz, h, :],
                                rhs=ex[0:msz, EXOFF[si]:EXOFF[si] + NP],
                                start=(si == 0), stop=(si == 2))
                    nc.vector.tensor_copy(asba[b][0:65, h, :], pav[0:65, 0:NP])

            def emit_sc(b, qk, finb):
                asba[b] = asbp.tile([128, 16, NP], bf16, name="asb_all")
                if finb is not None:
                    att[finb] = atpool.tile([128, 8, NP], bf16, name="att")
                pend = None
                for p in range(8):
                    exs = [expool.tile([128, EXW], bf16, tag=f"e{hh}", name="ex")
                           for hh in range(2)]
                    slots = emit_scores(qk, p)
                    for hh in range(2):
                        emit_exp(slots[hh], exs[hh], hh)
                    if finb is not None:
                        for hh in range(2):
                            nc.vector.tensor_tensor(
                                att[finb][hh * 64:(hh + 1) * 64, p, :],
                                asba[finb][0:64, 2 * p + hh, :],
                                prb[finb][0:64, 2 * p + hh, :],
                                mybir.AluOpType.mult)
                    if pend is not None:
                        emit_av(b, *pend)
                    pend = (p, exs)
                emit_av(b, *pend)

            def emit_rec(b):
                staged = stpool.tile([16, NP], bf16, tag="staged", name="staged")
                nc.sync.dma_start(staged[:], asba[b][64:65, :, :])
                rec = stpool.tile([16, NP], bf16, tag="rec", name="rec")
                nc.vector.reciprocal(rec[:], staged[:])
                # bounce through DRAM, then broadcast each head's row across
                # 64 partitions with step-0 DMAs
                rd = drp.tile([16, NP], bf16, tag="rd", name="rd")
                nc.sync.dma_start(rd[:], rec[:])
                prb[b] = prbp.tile([64, 16, NP], bf16, name="prb")
                for h in range(16):
                    nc.sync.dma_start(prb[b][0:64, h, :],
                                      rd[h:h + 1, :].to_broadcast([64, NP]))

            def emit_fin(b):
                # standalone normalization (last batch)
                for p in range(8):
                    for hh in range(2):
                        nc.vector.tensor_tensor(
                            att[b][hh * 64:(hh + 1) * 64, p, :],
                            asba[b][0:64, 2 * p + hh, :],
                            prb[b][0:64, 2 * p + hh, :],
                            mybir.AluOpType.mult)

            def emit_proj(b):
                for s, (t0, tsz) in enumerate(MSL):
                    rsz = min(tsz, NTOK - t0)
                    ysb = yspool.tile([128, 1024], bf16, name="ysb")
                    for half in range(2):
                        py = hfp.tile([128, 512], f32, tag="hf", name="py")
                        for c in range(8):
                            nc.tensor.matmul(
                                py[0:tsz, :],
                                lhsT=att[b][:, c, t0:t0 + tsz],
                                rhs=wp[:, c, half * 512:(half + 1) * 512],
                                start=(c == 0), stop=(c == 7))
                        nc.scalar.copy(ysb[0:tsz, half * 512:(half + 1) * 512],
                                       py[0:tsz, :])
                    nc.sync.dma_start(
                        out_d[b * NTOK + t0: b * NTOK + t0 + rsz, :],
                        ysb[0:rsz, :])

            # ---- schedule ----
            emit_v(0); emit_v(1); emit_v(2)
            qk0 = emit_qk(0)
            emit_sc(0, qk0, None); emit_rec(0)
            emit_v(3)
            qk1 = emit_qk(1)
            emit_sc(1, qk1, 0); emit_rec(1)
            emit_proj(0)
            qk2 = emit_qk(2)
            emit_sc(2, qk2, 1); emit_rec(2)
            emit_proj(1)
            qk3 = emit_qk(3)
            emit_sc(3, qk3, 2); emit_rec(3)
            att[3] = atpool.tile([128, 8, NP], bf16, name="att3")
            emit_fin(3)
            emit_proj(2)
            emit_proj(3)
    nc.finalize()
    return nc


def _get_nc(mask_mode, use_qkv_bias):
    key = (mask_mode, use_qkv_bias)
    if key not in _cache:
        _cache[key] = _build(mask_mode, use_qkv_bias)
    return _cache[key]


def _bc_mask():
    seg = np.concatenate([np.full(s * s, i, dtype=np.int64) for i, s in enumerate(SCALES)])
    allow = seg[:, None] >= seg[None, :]
    return np.where(allow, 0.0, -1e9).astype(np.float32)[None, None]


def _prep_core_inputs(x, mask, qkv_w, qkv_b, proj_w, proj_b):
    tabs, consts = _host_tables()
    mf = mask.astype(np.float32)
    if not np.any(mf != 0):
        mask_mode = "none"
    elif np.array_equal(mf, _bc_mask()):
        mask_mode = "bc"
    else:
        mask_mode = "general"
    use_qb = bool(np.any(qkv_b != 0))

    wqkT = qkv_w.astype(np.float32).T                    # [1024, 3072]
    wv = np.ascontiguousarray(wqkT[:, 2048:].astype(BF16))
    wpT = np.ascontiguousarray(proj_w.astype(np.float32).T.astype(BF16))

    common = {"wv": wv, "wp": wpT, "tabs": np.ascontiguousarray(tabs),
              "consts": np.ascontiguousarray(consts)}
    if FP8_QK:
        common["wqk8"] = np.ascontiguousarray(
            (wqkT[:, :2048] * W8SCALE).astype(F8))
    else:
        common["wqk"] = np.ascontiguousarray(wqkT[:, :2048].astype(BF16))
    if mask_mode == "general":
        mT = mask[0, 0].astype(np.float32).T            # [keys, queries]
        mm = np.zeros((384, NP), np.float32)
        mm[:NTOK, :NTOK] = np.exp(mT)                   # multiplicative mask
        maskm = np.zeros((128, 3 * NP), np.float32)
        for s in range(3):
            maskm[:, s * NP:(s + 1) * NP] = mm[s * 128:(s + 1) * 128, :]
        common["maskm"] = maskm.astype(BF16)
    if use_qb:
        cos, sin = _rope_tables()
        sin2 = sin.copy(); sin2[:, 0::2] = -sin[:, 0::2]
        scale = 1.0 / np.sqrt(HD)
        qb_full = np.zeros((128, 16 * NP), np.float32)
        bq = qkv_b[:2048].astype(np.float32)
        for f in range(16):
            is_q = f < 8
            sc = scale if is_q else 1.0
            for hh in range(2):
                hvec = bq[f * 128 + hh * 64: f * 128 + (hh + 1) * 64]  # [64]
                hswap = hvec.reshape(-1, 2)[:, ::-1].reshape(-1)
                rb = cos * hvec[None, :] + sin2 * hswap[None, :]       # [341,64]
                qb_full[hh * 64:(hh + 1) * 64, f * NP: f * NP + NTOK] = sc * rb.T
        common["qb"] = qb_full.astype(BF16)
        common["vb"] = qkv_b[2048:].astype(np.float32).astype(BF16)[None, :]

    in_maps = []
    xf = x.astype(np.float32)
    for core in range(NCORES):
        xc = xf[core * BPC:(core + 1) * BPC]            # [4, 341, 1024]
        xp = np.zeros((BPC, NP, DIM), np.float32)
        xp[:, :NTOK, :] = xc
        xT = np.ascontiguousarray(xp.reshape(BPC * NP, DIM).T)  # [1024, 1376]
        m = dict(common)
        m["xt"] = xT.astype(BF16)
        if FP8_QK:
            m["xt8"] = xT.astype(F8)
        in_maps.append(m)
    return in_maps, mask_mode, use_qb


def kernel(x, mask, qkv_w, qkv_b, proj_w, proj_b, _trace=False):
    from concourse.bass_utils import run_bass_kernel_spmd
    x, mask, qkv_w, qkv_b, proj_w, proj_b = (
        np.asarray(t) for t in (x, mask, qkv_w, qkv_b, proj_w, proj_b))
    in_maps, mask_mode, use_qb = _prep_core_inputs(
        x, mask, qkv_w, qkv_b, proj_w, proj_b)
    nc = _get_nc(mask_mode, use_qb)
    res = run_bass_kernel_spmd(nc, in_maps, core_ids=list(range(NCORES)),
                               trace=_trace)
    out = np.empty((B, NTOK, DIM), np.float32)
    for core in range(NCORES):
        y = res.results[core]["out"].astype(np.float32).reshape(BPC, NTOK, DIM)
        out[core * BPC:(core + 1) * BPC] = y
    pb = proj_b.astype(np.float32)
    if np.any(pb != 0):
        out += pb[None, None, :]
    kernel._last_exec_time_ns = res.exec_time_ns
    kernel._last_results = res
    return out
